# revision 36
# baseline (speedup 1.0000x reference)
"""Trainium2 Bass kernel for nn_NodeAttnModel (GATv2Conv + norm + MLP).

v2 architecture — no xl table, no dma_gather.

Key ideas:
  - Edges are sorted by destination and grouped into 80-node dst blocks;
    each 128-edge chunk belongs to one block.
  - The per-edge pre-activation  s = xl[src] + xr[dst] + ea@We  is produced
    by exactly TWO matmuls per chunk:
        MM1:  lhsT = Xp_hi (x[src].T rows 0:128, staged on host)  rhs = Wl_hi
        MM2:  lhsT = [Xp_lo(32); eaT(16); onehot_dst(80)]
              rhs  = [Wl_lo   ; We     ; xr_block     ]        (K = 128)
    The host supplies x[src] already permuted into edge order (it is a
    permutation of an *input*, so no on-device gather is needed).
  - Aggregation uses softmax linearity to avoid needing xl[src] per edge:
        agg = sum_e alpha*xl[src] = (S_s - S_ea@We5)/den - xr * den/(den+eps)
    where S_s = sum_e exp*s, S_ea = sum_e exp*ea (80 cols: 5 heads x 16),
    den = sum_e exp, all produced by ONE scatter matmul per chunk
    (rhs = [exp*s | exp*ea | exp], 245 cols, lhsT = dst one-hot).
  - Node phase: correction matmul + gating, then residual/LN/MLP as before.

All DMAs are large and batched (GROUP=8 tiles per call) to stay off the
descriptor-rate limits that dominated v1.
"""

import math

import numpy as np
import ml_dtypes

import concourse.bass as bass
import concourse.bacc as bacc
import concourse.mybir as mybir
import concourse.tile as tile
from concourse.bass_utils import run_bass_kernel_spmd

BF = ml_dtypes.bfloat16
F32 = mybir.dt.float32
BF16 = mybir.dt.bfloat16
AL = mybir.AluOpType
AF = mybir.ActivationFunctionType

# Problem constants
N, D, H, C, E, ED, HID = 50000, 160, 5, 32, 800000, 16, 512
EPS = 1e-5
SLOPE = 0.2
SELU_L = 1.0507009873554805
SELU_A = 1.6732632423543772

NCORES = 8
P = 128
BLK = 64           # dst nodes per block (mask rows 64:128 of the MM2 lhsT)
KLO = 48           # Xp_lo(32) + eaT(16) rows of the MM2 lhsT
KLOD = 64          # xlo DMA rows (48 data + 16 host zeros for K alignment)
CPT = 8            # chunks (of 128 edges) per tile
GROUP = 8          # tiles per DMA group
SENT = 1000.0      # dst_rel sentinel for padding edges
W_S = D            # wsea columns: [0:160) exp*s
W_EA = D + H * ED  # [160:240) exp*ea
W_DEN = W_EA + H   # [240:245) exp ; padded to 248
W_PAD = 248


class Cfg:
    def __init__(self, n=N, e=E, ncores=NCORES):
        self.N, self.E, self.NCORES = n, e, ncores
        self.NV = n // ncores                      # nodes per core
        self.NBLK = math.ceil(self.NV / BLK)       # dst blocks per core
        self.NPAD = self.NBLK * BLK                # block-padded nodes
        self.NT = math.ceil(self.NPAD / P)         # node-phase tiles
        self.NTP = self.NT * P                     # 128-padded nodes
        self.G = None                              # set after edge prep


def _prep_edges(cfg, x, edge_index, edge_attr):
    """Sort/pad edges, stage the permuted x[src] and edge data per core."""
    src = np.asarray(edge_index[0]).astype(np.int64)
    dst = np.asarray(edge_index[1]).astype(np.int64)
    e = src.shape[0]
    core = dst // cfg.NV
    rel = dst - core * cfg.NV
    blk = rel // BLK
    lane = rel - blk * BLK
    gkey = core * cfg.NBLK + blk
    order = np.argsort(gkey, kind="stable")
    gcounts = np.bincount(gkey, minlength=cfg.NCORES * cfg.NBLK)
    counts = gcounts.reshape(cfg.NCORES, cfg.NBLK)
    chunks_per = -(-counts.max(axis=0) // P)       # [NBLK]
    chunks_per[chunks_per == 0] = 1
    S = int(chunks_per.sum())
    T = -(-S // CPT)
    G = -(-T // GROUP)
    T = G * GROUP
    S_pad = T * CPT
    cfg.G = G

    chunk_blk = np.full(S_pad, cfg.NBLK - 1, np.int64)
    chunk_base = np.zeros(cfg.NBLK, np.int64)
    pos = 0
    for b in range(cfg.NBLK):
        chunk_base[b] = pos
        nch = int(chunks_per[b])
        chunk_blk[pos:pos + nch] = b
        pos += nch
    first_chunk = np.zeros(cfg.NBLK, np.int64)
    last_chunk = np.zeros(cfg.NBLK, np.int64)
    for b in range(cfg.NBLK):
        w = np.nonzero(chunk_blk == b)[0]
        first_chunk[b], last_chunk[b] = w[0], w[-1]

    gstart = np.zeros_like(gcounts)
    gstart[1:] = np.cumsum(gcounts)[:-1]
    ranks = np.arange(e) - gstart[gkey[order]]
    ecore = core[order]
    eslot = chunk_base[blk[order]] * P + ranks       # slot in [0, S_pad*P)

    SL = S_pad * P
    xbf = np.asarray(x, np.float32).astype(BF)
    ea32 = np.asarray(edge_attr, np.float32).astype(BF)

    Xflat = np.zeros((cfg.NCORES, SL, D), BF)
    EAflat = np.zeros((cfg.NCORES, SL, ED), BF)
    drel_flat = np.full((cfg.NCORES, SL), SENT, np.float32)
    Xflat[ecore, eslot] = xbf[src[order]]
    EAflat[ecore, eslot] = ea32[order]
    drel_flat[ecore, eslot] = lane[order].astype(np.float32)

    # [NC, G, 8192, D] -> feature-major per group
    Xg = Xflat.reshape(cfg.NCORES, G, GROUP * CPT * P, D)
    xph = np.ascontiguousarray(Xg[:, :, :, 0:P].transpose(0, 1, 3, 2))
    xlo = np.zeros((cfg.NCORES, G, KLOD, GROUP * CPT * P), BF)
    xlo[:, :, 0:D - P, :] = Xg[:, :, :, P:D].transpose(0, 1, 3, 2)
    EAg = EAflat.reshape(cfg.NCORES, G, GROUP * CPT * P, ED)
    xlo[:, :, D - P:KLO, :] = EAg.transpose(0, 1, 3, 2)
    # edge-major ea rows: [NC, G, 128, GROUP*CPT*ED]
    ear = np.ascontiguousarray(
        EAg.reshape(cfg.NCORES, G, GROUP * CPT, P, ED).transpose(0, 1, 3, 2, 4)
        .reshape(cfg.NCORES, G, P, GROUP * CPT * ED))
    dg = drel_flat.reshape(cfg.NCORES, G, GROUP * CPT, P)
    drelT = np.ascontiguousarray(
        dg.transpose(0, 1, 3, 2)).astype(BF)       # [NC, G, 128, G*CPT]
    drow = np.ascontiguousarray(
        dg.reshape(cfg.NCORES, G, 1, GROUP * CPT * P)).astype(BF)

    sched = dict(
        T=T, G=G,
        chunk_blk=chunk_blk.tolist(),
        first_chunk=first_chunk.tolist(),
        last_chunk=last_chunk.tolist(),
    )
    return sched, xph, xlo, ear, drelT, drow


def _nontriv(a, v):
    return not np.all(np.asarray(a) == v)


def build_trace(cfg, sched, weights):
    G = sched["G"]
    chunk_blk = sched["chunk_blk"]
    first_chunk_of = {g: b for b, g in enumerate(sched["first_chunk"])}
    last_chunk_of = {g: b for b, g in enumerate(sched["last_chunk"])}

    W = weights
    use_bl = _nontriv(W["bl"], 0.0)
    use_br = _nontriv(W["br"], 0.0)
    use_bgat = _nontriv(W["b_gat"], 0.0)
    use_g1 = _nontriv(W["g1"], 1.0)
    use_b1 = _nontriv(W["b1"], 0.0)
    use_bm1 = _nontriv(W["b_m1"], 0.0)
    use_gm = _nontriv(W["g_m"], 1.0)
    use_bm = _nontriv(W["b_m"], 0.0)
    use_bm2 = _nontriv(W["b_m2"], 0.0)
    use_g2 = _nontriv(W["g2"], 1.0)
    use_b2 = _nontriv(W["b2"], 0.0)

    nc = bacc.Bacc("TRN2", target_bir_lowering=False, debug=False)

    NBLK, NT, NTP = cfg.NBLK, cfg.NT, cfg.NTP
    EPG = GROUP * CPT * P          # edges per group (8192)
    CPG = GROUP * CPT              # chunks per group (64)

    # ---------------- I/O declarations ----------------
    d_xph = nc.dram_tensor("xph", [G, P, EPG], BF16, kind="ExternalInput")
    d_xlo = nc.dram_tensor("xlo", [G, KLOD, EPG], BF16, kind="ExternalInput")
    d_ear = nc.dram_tensor("ear", [G, P, CPG * ED], BF16, kind="ExternalInput")
    d_drelT = nc.dram_tensor("drelT", [G, P, CPG], BF16, kind="ExternalInput")
    d_drow = nc.dram_tensor("drow", [G, 1, EPG], BF16, kind="ExternalInput")
    d_xoT_hi = nc.dram_tensor("xoT_hi", [P, NTP], BF16, kind="ExternalInput")
    d_xoT_lo = nc.dram_tensor("xoT_lo", [D - P, NTP], BF16, kind="ExternalInput")
    d_xown = nc.dram_tensor("x_own", [NTP, D], F32, kind="ExternalInput")
    d_out = nc.dram_tensor("y_out", [NTP, D], F32, kind="ExternalOutput")

    def inline(arr, name):
        return nc.inline_tensor(np.ascontiguousarray(arr), name=name)

    bf = lambda a: np.asarray(a, np.float32).astype(BF)
    Wl = np.asarray(W["Wl"], np.float32)
    We = np.asarray(W["We"], np.float32)
    c_Wl_hi = inline(bf(Wl[0:P, :]), "c_Wl_hi")
    # MM2 rhs top 48 rows: [Wl_lo; We]
    rhs_lo = np.concatenate([Wl[P:D, :], We], axis=0)       # [48, 160]
    c_rhs_lo = inline(bf(rhs_lo), "c_rhs_lo")
    c_Wr_hi = inline(bf(W["Wr"][0:P, :]), "c_Wr_hi")
    c_Wr_lo = inline(bf(W["Wr"][P:D, :]), "c_Wr_lo")
    # We5: block-diagonal per-head We  [80, 160]
    We5 = np.zeros((H * ED, D), np.float32)
    for h in range(H):
        We5[h * ED:(h + 1) * ED, h * C:(h + 1) * C] = We[:, h * C:(h + 1) * C]
    c_We5 = inline(bf(We5), "c_We5")
    c_ident = inline(np.eye(P, dtype=BF), "c_ident")
    c_att = inline(np.broadcast_to(
        bf(np.asarray(W["att"]).reshape(1, D)), (P, D)).copy(), "c_att")
    c_iota_c = inline(np.arange(BLK, dtype=np.float32).reshape(BLK, 1),
                      "c_iota_c")
    c_iota_r = inline(np.broadcast_to(
        np.arange(BLK, dtype=np.float32).reshape(1, BLK).astype(BF),
        (P, BLK)).copy(), "c_iota_r")
    c_Wm1_hi = inline(bf(W["W_m1"][0:P, :]), "c_Wm1_hi")
    c_Wm1_lo = inline(bf(W["W_m1"][P:D, :]), "c_Wm1_lo")
    c_Wm2 = inline(
        bf(W["W_m2"]).reshape(4, P, D).transpose(1, 0, 2).copy(), "c_Wm2")
    rows32 = lambda a: np.broadcast_to(
        np.asarray(a, np.float32).reshape(1, -1), (P, np.asarray(a).size)).copy()
    c_bl = inline(rows32(W["bl"]), "c_bl")
    c_br = inline(rows32(W["br"]), "c_br")
    c_bgat = inline(rows32(W["b_gat"]), "c_bgat")
    c_g1 = inline(rows32(W["g1"]), "c_g1")
    c_b1 = inline(rows32(W["b1"]), "c_b1")
    c_bm1 = inline(rows32(W["b_m1"]), "c_bm1")
    c_gm = inline(rows32(W["g_m"]), "c_gm")
    c_bm = inline(rows32(W["b_m"]), "c_bm")
    c_bm2 = inline(rows32(W["b_m2"]), "c_bm2")
    c_g2 = inline(rows32(W["g2"]), "c_g2")
    c_b2 = inline(rows32(W["b2"]), "c_b2")

    with tile.TileContext(nc) as tc:
        mesh_scope, _ = nc.enter_named_scope("ALGO_MESH", True)
        psp = tc.alloc_tile_pool(name="psp", bufs=8, space="PSUM")
        dram = tc.alloc_tile_pool(name="dram", bufs=1, space="DRAM")
        agg_d = dram.tile([NTP, W_PAD], F32, name="agg_d", tag="agg_d")
        xr_d = dram.tile([NTP, D], BF16, name="xr_d", tag="xr_d")

        cp = tc.alloc_tile_pool(name="consts", bufs=1)

        def csb(dr, shape, dtype, name):
            t = cp.tile(shape, dtype, name=name, tag=name)
            nc.sync.dma_start(out=t[tuple(slice(0, s) for s in shape)], in_=dr[:])
            return t

        Wl_hi = csb(c_Wl_hi, [P, D], BF16, "Wl_hi")
        rhs_lo_sb = csb(c_rhs_lo, [KLO, D], BF16, "rhs_lo_sb")
        Wr_hi = csb(c_Wr_hi, [P, D], BF16, "Wr_hi")
        Wr_lo = csb(c_Wr_lo, [D - P, D], BF16, "Wr_lo")
        We5_sb = csb(c_We5, [H * ED, D], BF16, "We5_sb")
        ident = csb(c_ident, [P, P], BF16, "ident")
        att_sb = csb(c_att, [P, D], BF16, "att_sb")
        iota_c = csb(c_iota_c, [BLK, 1], F32, "iota_c")
        iota_r = csb(c_iota_r, [P, BLK], BF16, "iota_r")
        Wm1_hi = csb(c_Wm1_hi, [P, HID], BF16, "Wm1_hi")
        Wm1_lo = csb(c_Wm1_lo, [D - P, HID], BF16, "Wm1_lo")
        Wm2_sb = csb(c_Wm2, [P, 4, D], BF16, "Wm2_sb")
        bl_sb = csb(c_bl, [P, D], F32, "bl_sb")
        br_sb = csb(c_br, [P, D], F32, "br_sb")
        bgat_sb = csb(c_bgat, [P, D], F32, "bgat_sb")
        g1_sb = csb(c_g1, [P, D], F32, "g1_sb")
        b1_sb = csb(c_b1, [P, D], F32, "b1_sb")
        bm1_sb = csb(c_bm1, [P, HID], F32, "bm1_sb")
        gm_sb = csb(c_gm, [P, HID], F32, "gm_sb")
        bm_sb = csb(c_bm, [P, HID], F32, "bm_sb")
        bm2_sb = csb(c_bm2, [P, D], F32, "bm2_sb")
        g2_sb = csb(c_g2, [P, D], F32, "g2_sb")
        b2_sb = csb(c_b2, [P, D], F32, "b2_sb")
        eps_sb = cp.tile([P, 1], F32, name="eps_sb", tag="eps_sb")
        nc.gpsimd.memset(eps_sb[:, :], float(EPS))
        lna_sb = cp.tile([P, 1], F32, name="lna_sb", tag="lna_sb")
        nc.gpsimd.memset(lna_sb[:, :], float(math.log(SELU_L * SELU_A)))

        # xoT kept resident: phase A (xr blocks) + phase C (xr recompute)
        xoT_hi = cp.tile([P, NTP], BF16, name="xoT_hi", tag="xoT_hi")
        nc.sync.dma_start(out=xoT_hi[:, :], in_=d_xoT_hi[:])
        xoT_lo = cp.tile([D - P, NTP], BF16, name="xoT_lo", tag="xoT_lo")
        nc.sync.dma_start(out=xoT_lo[:, :], in_=d_xoT_lo[:])

        # MM2 rhs per block: [Wl_lo; We; 0; xr_b]
        rhs_all = cp.tile([P, NBLK * D], BF16, name="rhs_all", tag="rhs_all")

        # ---------------- phase A: xr per block ----------------
        sc_xr, _ = nc.enter_named_scope("pA_xr", True)
        nc.gpsimd.memset(rhs_all[32:KLOD, :], 0.0)
        xr_sb = None
        for b in range(NBLK):
            nc.scalar.copy(out=rhs_all[0:KLO, b * D:(b + 1) * D],
                           in_=rhs_lo_sb[:, :])
            ps = psp.tile([BLK, D], F32, name=f"ps_xr{b}", tag="ps")
            cs = slice(b * BLK, (b + 1) * BLK)
            nc.tensor.matmul(ps[:, :], xoT_hi[:, cs], Wr_hi[:, :],
                             start=True, stop=False)
            nc.tensor.matmul(ps[:, :], xoT_lo[:, cs], Wr_lo[:, :],
                             start=False, stop=True)
            dst = rhs_all[KLOD:P, b * D:(b + 1) * D]
            if use_br:
                nc.vector.tensor_tensor(out=dst, in0=ps[:, :],
                                        in1=br_sb[0:BLK, :], op=AL.add)
            else:
                nc.vector.tensor_scalar_add(out=dst, in0=ps[:, :], scalar1=0.0)
            # stash xr rows for the node phase (incl. br if present)
            if xr_sb is None:
                xr_lo = b
                xr_sb = cp.tile([BLK, 8, D], BF16, name=f"xrsb{b}",
                                tag="xrsb", bufs=2)
            if use_br:
                nc.gpsimd.tensor_tensor(out=xr_sb[:, b - xr_lo, :],
                                        in0=ps[:, :], in1=br_sb[0:BLK, :],
                                        op=AL.add)
            else:
                nc.scalar.copy(out=xr_sb[:, b - xr_lo, :], in_=ps[:, :])
            if b - xr_lo == 7 or b == NBLK - 1:
                nb = b - xr_lo + 1
                nc.scalar.dma_start(
                    out=xr_d[xr_lo * BLK:(b + 1) * BLK, :]
                    .rearrange("(t l) w -> l t w", l=BLK),
                    in_=xr_sb[:, 0:nb, :])
                xr_sb = None
        nc.leave_named_scope("pA_xr", sc_xr, True)

        tc.strict_bb_all_engine_barrier()

        # ---------------- phase B: edges ----------------
        ep = tc.alloc_tile_pool(name="ep", bufs=1)
        sc_ed, _ = nc.enter_named_scope("pB_edge", True)
        # PE warm-up: sustained busy window pushes HAM to full clock
        prime = psp.tile([P, HID], F32, name="prime_b", tag="ps")
        for i in range(16):
            nc.tensor.matmul(prime[:, :], ident[:, :], Wm1_hi[:, :],
                             start=(i == 0), stop=(i == 15))
        agg_tiles = {}
        agg_sb = None
        agg_lo = 0
        for g in range(G):
            xph_t = ep.tile([P, EPG], BF16, name=f"xph{g}", tag="xph", bufs=2)
            comb_t = ep.tile([P, EPG], BF16, name=f"comb{g}", tag="comb", bufs=2)
            ear_t = ep.tile([P, CPG * ED], BF16, name=f"ear{g}", tag="ear",
                            bufs=2)
            drelT_t = ep.tile([P, CPG], BF16, name=f"drelT{g}", tag="drelT",
                              bufs=2)
            nc.sync.dma_start(out=xph_t[:, :], in_=d_xph[g])
            nc.sync.dma_start(out=comb_t[0:KLOD, :], in_=d_xlo[g])
            nc.sync.dma_start(out=ear_t[:, :], in_=d_ear[g])
            nc.sync.dma_start(out=drelT_t[:, :], in_=d_drelT[g])
            nc.sync.dma_start(out=comb_t[KLOD:P, :],
                              in_=d_drow[g].to_broadcast([BLK, EPG]))

            # one-hot masks: comb rows 64:128 (pick, in place), m2 (scatter)
            nc.vector.tensor_scalar(
                out=comb_t[KLOD:P, :], in0=comb_t[KLOD:P, :],
                scalar1=iota_c[:, 0:1], scalar2=None, op0=AL.is_equal)
            m2 = ep.tile([P, CPG, BLK], BF16, name=f"m2_{g}", tag="m2", bufs=2)
            nc.vector.tensor_tensor(
                out=m2[:, :, :],
                in0=iota_r[:, :].rearrange("p n -> p () n").to_broadcast(
                    [P, CPG, BLK]),
                in1=drelT_t[:, :].rearrange("p c -> p c ()").to_broadcast(
                    [P, CPG, BLK]),
                op=AL.is_equal)

            for t in range(GROUP):
                f_sb = ep.tile([P, CPT, D], BF16, name=f"f{g}_{t}", tag="f",
                               bufs=3)
                wsea = ep.tile([P, CPT, W_PAD], BF16, name=f"w{g}_{t}",
                               tag="w", bufs=3)
                sc_t = ep.tile([P, CPT, H], F32, name=f"sc{g}_{t}", tag="sc",
                               bufs=3)
                ps_list = []
                for k in range(CPT // 2):
                    ps_s = psp.tile([P, 2, D], F32, name=f"ps_s{g}_{t}_{k}",
                                    tag="ps")
                    for j in range(2):
                        c = t * CPT + 2 * k + j
                        gi = g * CPG + c
                        b = chunk_blk[gi]
                        es = slice(c * P, (c + 1) * P)
                        nc.tensor.matmul(ps_s[:, j, :], xph_t[:, es],
                                         Wl_hi[:, :], start=True, stop=False)
                        nc.tensor.matmul(ps_s[:, j, :], comb_t[:, es],
                                         rhs_all[:, b * D:(b + 1) * D],
                                         start=False, stop=True)
                    # f = leaky_relu(s), fused into the PSUM read
                    nc.scalar.activation(out=f_sb[:, 2 * k:2 * k + 2, :],
                                         in_=ps_s[:, :, :], func=AF.Prelu,
                                         alpha=float(SLOPE))
                    ps_list.append(ps_s)
                # f *= att (in place), per-head reduce, exp
                nc.vector.tensor_tensor(
                    out=f_sb[:, :, :], in0=f_sb[:, :, :],
                    in1=att_sb[:, :].rearrange("p f -> p () f").to_broadcast(
                        [P, CPT, D]),
                    op=AL.mult)
                nc.vector.tensor_reduce(
                    out=sc_t[:, :, :],
                    in_=f_sb[:, :, :].rearrange("p c (h z) -> p c h z", z=C),
                    axis=mybir.AxisListType.X, op=AL.add)
                nc.scalar.activation(out=wsea[:, :, W_EA:W_DEN],
                                     in_=sc_t[:, :, :], func=AF.Exp)
                expv = wsea[:, :, W_EA:W_DEN]
                for k in range(CPT // 2):
                    ks = slice(2 * k, 2 * k + 2)
                    nc.vector.tensor_tensor(
                        out=wsea[:, ks, 0:D].rearrange(
                            "p c (h z) -> p c h z", z=C),
                        in0=ps_list[k][:, :, :].rearrange(
                            "p c (h z) -> p c h z", z=C),
                        in1=expv[:, ks, :].rearrange(
                            "p c h -> p c h ()").to_broadcast([P, 2, H, C]),
                        op=AL.mult)
                nc.vector.tensor_tensor(
                    out=wsea[:, :, W_S:W_EA].rearrange(
                        "p c (h z) -> p c h z", z=ED),
                    in0=ear_t[:, :].rearrange(
                        "p (c z) -> p c () z", z=ED)[:, t * CPT:(t + 1) * CPT]
                    .to_broadcast([P, CPT, H, ED]),
                    in1=expv.rearrange("p c h -> p c h ()").to_broadcast(
                        [P, CPT, H, ED]),
                    op=AL.mult)

                for j in range(CPT):
                    c = t * CPT + j
                    gi = g * CPG + c
                    b = chunk_blk[gi]
                    if gi in first_chunk_of:
                        agg_tiles[b] = psp.tile([BLK, W_PAD], F32,
                                                name=f"agg{b}", tag="ps")
                    at = agg_tiles[b]
                    nc.tensor.matmul(at[:, :], m2[:, c, :], wsea[:, j, :],
                                     start=(gi in first_chunk_of),
                                     stop=(gi in last_chunk_of))
                    if gi in last_chunk_of:
                        if agg_sb is None:
                            agg_lo = b
                            agg_sb = ep.tile([BLK, 8, W_PAD], F32,
                                             name=f"aggsb{b}", tag="aggsb",
                                             bufs=2)
                        nc.scalar.copy(out=agg_sb[:, b - agg_lo, :],
                                       in_=at[:, :])
                        del agg_tiles[b]
                        if b - agg_lo == 7 or b == NBLK - 1:
                            nb = b - agg_lo + 1
                            nc.scalar.dma_start(
                                out=agg_d[agg_lo * BLK:(b + 1) * BLK, :]
                                .rearrange("(t l) w -> l t w", l=BLK),
                                in_=agg_sb[:, 0:nb, :])
                            agg_sb = None
        nc.leave_named_scope("pB_edge", sc_ed, True)

        tc.strict_bb_all_engine_barrier()
        ep.release()

        # ---------------- phase C: node phase ----------------
        npo = tc.alloc_tile_pool(name="np", bufs=1)
        sc_nd, _ = nc.enter_named_scope("pC_node", True)
        prime2 = psp.tile([P, HID], F32, name="prime_c", tag="ps")
        for i in range(16):
            nc.tensor.matmul(prime2[:, :], ident[:, :], Wm1_hi[:, :],
                             start=(i == 0), stop=(i == 15))
        NTG = 8
        for j0 in range(0, NT, NTG):
            nt = min(NTG, NT - j0)
            rs = slice(j0 * P, (j0 + nt) * P)
            x_g = npo.tile([P, NTG, D], F32, name=f"x{j0}", tag="x", bufs=2)
            a_g = npo.tile([P, NTG, W_PAD], F32, name=f"a{j0}", tag="a",
                           bufs=2)
            xr_g = npo.tile([P, NTG, D], BF16, name=f"xr{j0}", tag="xr",
                            bufs=2)
            nc.sync.dma_start(
                out=x_g[:, 0:nt, :],
                in_=d_xown[rs, :].rearrange("(t p) d -> p t d", p=P))
            nc.sync.dma_start(
                out=a_g[:, 0:nt, :],
                in_=agg_d[rs, :].rearrange("(t p) d -> p t d", p=P))
            nc.sync.dma_start(
                out=xr_g[:, 0:nt, :],
                in_=xr_d[rs, :].rearrange("(t p) d -> p t d", p=P))
            y_g = npo.tile([P, NTG, D], F32, name=f"y{j0}", tag="y", bufs=2)

            for jj in range(nt):
                j = j0 + jj
                x_t = x_g[:, jj, :]
                a_t = a_g[:, jj, :]
                cs = slice(j * P, (j + 1) * P)

                # correction: Sea @ We5
                sea_b = npo.tile([P, H * ED], BF16, name=f"seab{j}",
                                 tag="seab", bufs=2)
                nc.scalar.copy(out=sea_b[:, :], in_=a_t[:, W_S:W_EA])
                ps_t = psp.tile([H * ED, P], BF16, name=f"ps_t{j}", tag="ps")
                nc.tensor.transpose(out=ps_t[:, :], in_=sea_b[:, :],
                                    identity=ident[:, :])
                sea_T = npo.tile([H * ED, P], BF16, name=f"seaT{j}",
                                 tag="seaT", bufs=2)
                nc.scalar.copy(out=sea_T[:, :], in_=ps_t[:, :])
                ps_co = psp.tile([P, D], F32, name=f"ps_co{j}", tag="ps")
                nc.tensor.matmul(ps_co[:, :], sea_T[:, :], We5_sb[:, :],
                                 start=True, stop=True)

                # den, gate, reciprocal
                den = npo.tile([P, H], F32, name=f"den{j}", tag="den", bufs=2)
                nc.vector.tensor_scalar_add(out=den[:, :],
                                            in0=a_t[:, W_EA:W_DEN],
                                            scalar1=1e-30)
                rec = npo.tile([P, 2, H], F32, name=f"rec{j}", tag="rec",
                               bufs=2)
                nc.vector.reciprocal(out=rec[:, 0, :], in_=den[:, :])
                # gate = den_raw * rec ~= 1 (0 for isolated nodes)
                nc.vector.tensor_tensor(out=rec[:, 1, :],
                                        in0=a_t[:, W_EA:W_DEN],
                                        in1=rec[:, 0, :], op=AL.mult)

                # agg = (Ss - corr)*rec - xr*gate  (+x residual)
                t0 = npo.tile([P, D], F32, name=f"t0_{j}", tag="t0", bufs=2)
                nc.vector.tensor_tensor(out=t0[:, :], in0=a_t[:, 0:D],
                                        in1=ps_co[:, :], op=AL.subtract)
                nc.vector.tensor_tensor(
                    out=t0[:, :].rearrange("p (h z) -> p h z", z=C),
                    in0=t0[:, :].rearrange("p (h z) -> p h z", z=C),
                    in1=rec[:, 0, :].rearrange("p h -> p h ()").to_broadcast(
                        [P, H, C]),
                    op=AL.mult)
                xrg = npo.tile([P, D], F32, name=f"xrg{j}", tag="xrg", bufs=2)
                nc.vector.tensor_tensor(
                    out=xrg[:, :].rearrange("p (h z) -> p h z", z=C),
                    in0=xr_g[:, jj, :].rearrange("p (h z) -> p h z", z=C),
                    in1=rec[:, 1, :].rearrange("p h -> p h ()").to_broadcast(
                        [P, H, C]),
                    op=AL.mult)
                nc.vector.tensor_tensor(out=t0[:, :], in0=t0[:, :],
                                        in1=xrg[:, :], op=AL.subtract)
                nc.vector.tensor_tensor(out=t0[:, :], in0=x_t[:, :],
                                        in1=t0[:, :], op=AL.add)
                if use_bl:
                    nc.gpsimd.tensor_tensor(out=t0[:, :], in0=t0[:, :],
                                            in1=bl_sb[:, :], op=AL.add)
                if use_bgat:
                    nc.gpsimd.tensor_tensor(out=t0[:, :], in0=t0[:, :],
                                            in1=bgat_sb[:, :], op=AL.add)

                def layer_norm(src_ap, width, g_sb, b_sb, use_g, use_b,
                               out_dtype, ph, nm):
                    st = npo.tile([P, 6], F32, name=f"st{nm}", tag=f"st{ph}",
                                  bufs=2)
                    nc.vector.bn_stats(out=st[:, :], in_=src_ap)
                    mv = npo.tile([P, 2], F32, name=f"mv{nm}", tag=f"mv{ph}",
                                  bufs=2)
                    nc.vector.bn_aggr(out=mv[:, :], in_=st[:, :])
                    sd = npo.tile([P, 1], F32, name=f"sd{nm}", tag=f"sd{ph}",
                                  bufs=2)
                    nc.scalar.activation(out=sd[:, :], in_=mv[:, 1:2],
                                         func=AF.Ln, bias=eps_sb[:, 0:1])
                    rstd = npo.tile([P, 1], F32, name=f"rstd{nm}",
                                    tag=f"rstd{ph}", bufs=2)
                    nc.scalar.activation(out=rstd[:, :], in_=sd[:, :],
                                         func=AF.Exp, scale=-0.5)
                    o = npo.tile([P, width], out_dtype, name=f"ln{nm}",
                                 tag=f"ln{ph}", bufs=2)
                    nc.vector.scalar_tensor_tensor(
                        out=o[:, :], in0=src_ap, scalar=mv[:, 0:1],
                        in1=rstd[:, 0:1].to_broadcast([P, width]),
                        op0=AL.subtract, op1=AL.mult)
                    if use_g:
                        nc.vector.tensor_tensor(out=o[:, :], in0=o[:, :],
                                                in1=g_sb[:, :], op=AL.mult)
                    if use_b:
                        nc.gpsimd.tensor_tensor(out=o[:, :], in0=o[:, :],
                                                in1=b_sb[:, :], op=AL.add)
                    return o

                out1 = layer_norm(t0[:, :], D, g1_sb, b1_sb, use_g1, use_b1,
                                  F32, 1, f"1_{j}")
                out1b = npo.tile([P, D], BF16, name=f"o1b{j}", tag="o1b",
                                 bufs=2)
                nc.scalar.copy(out=out1b[:, :], in_=out1[:, :])

                pt0 = psp.tile([P, P], BF16, name=f"pt0_{j}", tag="ps")
                nc.tensor.transpose(out=pt0[:, :], in_=out1b[:, 0:P],
                                    identity=ident[:, :])
                t0s = npo.tile([P, P], BF16, name=f"t0s{j}", tag="t0s", bufs=2)
                nc.scalar.copy(out=t0s[:, :], in_=pt0[:, :])
                pt1 = psp.tile([D - P, P], BF16, name=f"pt1_{j}", tag="ps")
                nc.tensor.transpose(out=pt1[:, :], in_=out1b[:, P:D],
                                    identity=ident[:, :])
                t1s = npo.tile([D - P, P], BF16, name=f"t1s{j}", tag="t1s",
                               bufs=2)
                nc.scalar.copy(out=t1s[:, :], in_=pt1[:, :])
                ps_h = psp.tile([P, HID], F32, name=f"ps_h{j}", tag="ps")
                nc.tensor.matmul(ps_h[:, :], t0s[:, :], Wm1_hi[:, :],
                                 start=True, stop=False)
                nc.tensor.matmul(ps_h[:, :], t1s[:, :], Wm1_lo[:, :],
                                 start=False, stop=True)

                if use_bm1:
                    y_sb = npo.tile([P, HID], F32, name=f"ysb{j}", tag="ysb",
                                    bufs=2)
                    nc.vector.tensor_tensor(out=y_sb[:, :], in0=ps_h[:, :],
                                            in1=bm1_sb[:, :], op=AL.add)
                    ysrc = y_sb[:, :]
                else:
                    ysrc = ps_h[:, :]
                e_sb = npo.tile([P, HID], BF16, name=f"esb{j}", tag="esb",
                                bufs=2)
                nc.scalar.activation(out=e_sb[:, :], in_=ysrc, func=AF.Exp,
                                     bias=lna_sb[:, 0:1])
                r_sb = npo.tile([P, HID], BF16, name=f"rsb{j}", tag="rsb",
                                bufs=2)
                nc.scalar.activation(out=r_sb[:, :], in_=ysrc, func=AF.Relu,
                                     scale=float(SELU_L))
                u2 = npo.tile([P, HID], BF16, name=f"u2_{j}", tag="u2", bufs=2)
                nc.vector.scalar_tensor_tensor(
                    out=u2[:, :], in0=e_sb[:, :], scalar=float(SELU_L * SELU_A),
                    in1=r_sb[:, :], op0=AL.min, op1=AL.add)

                h_bf = layer_norm(u2[:, :], HID, gm_sb, bm_sb, use_gm, use_bm,
                                  BF16, 2, f"2_{j}")

                ps_m = psp.tile([P, D], F32, name=f"ps_m{j}", tag="ps")
                for k in range(4):
                    pth = psp.tile([P, P], BF16, name=f"pth{j}_{k}", tag="ps")
                    nc.tensor.transpose(out=pth[:, :],
                                        in_=h_bf[:, k * P:(k + 1) * P],
                                        identity=ident[:, :])
                    hts = npo.tile([P, P], BF16, name=f"hts{j}_{k}", tag="hts",
                                   bufs=3)
                    nc.scalar.copy(out=hts[:, :], in_=pth[:, :])
                    nc.tensor.matmul(ps_m[:, :], hts[:, :], Wm2_sb[:, k, :],
                                     start=(k == 0), stop=(k == 3))

                t2 = npo.tile([P, D], F32, name=f"t2_{j}", tag="t2", bufs=2)
                nc.vector.tensor_tensor(out=t2[:, :], in0=out1[:, :],
                                        in1=ps_m[:, :], op=AL.add)
                if use_bm2:
                    nc.gpsimd.tensor_tensor(out=t2[:, :], in0=t2[:, :],
                                            in1=bm2_sb[:, :], op=AL.add)
                y = layer_norm(t2[:, :], D, g2_sb, b2_sb, use_g2, use_b2, F32,
                               3, f"3_{j}")
                nc.vector.tensor_scalar_add(out=y_g[:, jj, :], in0=y[:, :],
                                            scalar1=0.0)
            nc.scalar.dma_start(
                out=d_out[rs, :].rearrange("(t p) d -> p t d", p=P),
                in_=y_g[:, 0:nt, :])
        nc.leave_named_scope("pC_node", sc_nd, True)

        npo.release()
        psp.release()
        dram.release()
        cp.release()
        nc.leave_named_scope("ALGO_MESH", mesh_scope, True)

    nc.compile()
    return nc


def _make_in_maps(cfg, x, xph, xlo, ear, drelT, drow):
    x32 = np.asarray(x, np.float32)
    in_maps = []
    for k in range(cfg.NCORES):
        xo = np.zeros((cfg.NTP, D), np.float32)
        xo[:cfg.NV] = x32[k * cfg.NV:(k + 1) * cfg.NV]
        xoT = np.zeros((D, cfg.NTP), BF)
        xoT[:, :cfg.NV] = x32[k * cfg.NV:(k + 1) * cfg.NV].astype(BF).T
        in_maps.append({
            "xph": xph[k], "xlo": xlo[k], "ear": ear[k],
            "drelT": drelT[k], "drow": drow[k],
            "xoT_hi": np.ascontiguousarray(xoT[0:P]),
            "xoT_lo": np.ascontiguousarray(xoT[P:D]),
            "x_own": xo,
        })
    return in_maps


def build_all(inputs, cfg=None):
    cfg = cfg or Cfg()
    sched, xph, xlo, ear, drelT, drow = _prep_edges(
        cfg, inputs["x"], inputs["edge_index"], inputs["edge_attr"])
    wnames = ["Wl", "bl", "Wr", "br", "We", "att", "b_gat", "g1", "b1",
              "W_m1", "b_m1", "g_m", "b_m", "W_m2", "b_m2", "g2", "b2"]
    weights = {k: np.asarray(inputs[k], np.float32) for k in wnames}
    nc = build_trace(cfg, sched, weights)
    in_maps = _make_in_maps(cfg, inputs["x"], xph, xlo, ear, drelT, drow)
    return cfg, nc, in_maps


def kernel(**inputs) -> np.ndarray:
    cfg, nc, in_maps = build_all(inputs)
    res = run_bass_kernel_spmd(nc, in_maps, core_ids=list(range(cfg.NCORES)))
    out = np.concatenate(
        [r["y_out"][:cfg.NV] for r in res.results], axis=0
    ).astype(np.float32)
    return out


# revision 38
# speedup vs baseline: 1.5463x; 1.5463x over previous
"""Trainium2 Bass kernel for nn_NodeAttnModel (GATv2Conv + norm + MLP).

v2 architecture — no xl table, no dma_gather.

Key ideas:
  - Edges are sorted by destination and grouped into 80-node dst blocks;
    each 128-edge chunk belongs to one block.
  - The per-edge pre-activation  s = xl[src] + xr[dst] + ea@We  is produced
    by exactly TWO matmuls per chunk:
        MM1:  lhsT = Xp_hi (x[src].T rows 0:128, staged on host)  rhs = Wl_hi
        MM2:  lhsT = [Xp_lo(32); eaT(16); onehot_dst(80)]
              rhs  = [Wl_lo   ; We     ; xr_block     ]        (K = 128)
    The host supplies x[src] already permuted into edge order (it is a
    permutation of an *input*, so no on-device gather is needed).
  - Aggregation uses softmax linearity to avoid needing xl[src] per edge:
        agg = sum_e alpha*xl[src] = (S_s - S_ea@We5)/den - xr * den/(den+eps)
    where S_s = sum_e exp*s, S_ea = sum_e exp*ea (80 cols: 5 heads x 16),
    den = sum_e exp, all produced by ONE scatter matmul per chunk
    (rhs = [exp*s | exp*ea | exp], 245 cols, lhsT = dst one-hot).
  - Node phase: correction matmul + gating, then residual/LN/MLP as before.

All DMAs are large and batched (GROUP=8 tiles per call) to stay off the
descriptor-rate limits that dominated v1.
"""

import math

import numpy as np
import ml_dtypes

import concourse.bass as bass
import concourse.bacc as bacc
import concourse.mybir as mybir
import concourse.tile as tile
from concourse.bass_utils import run_bass_kernel_spmd

BF = ml_dtypes.bfloat16
F32 = mybir.dt.float32
BF16 = mybir.dt.bfloat16
AL = mybir.AluOpType
AF = mybir.ActivationFunctionType

# Problem constants
N, D, H, C, E, ED, HID = 50000, 160, 5, 32, 800000, 16, 512
EPS = 1e-5
SLOPE = 0.2
SELU_L = 1.0507009873554805
SELU_A = 1.6732632423543772

NCORES = 8
P = 128
BLK = 64           # dst nodes per block (mask rows 64:128 of the MM2 lhsT)
KLO = 48           # Xp_lo(32) + eaT(16) rows of the MM2 lhsT
KLOD = 64          # xlo DMA rows (48 data + 16 host zeros for K alignment)
CPT = 8            # chunks (of 128 edges) per tile
GROUP = 8          # tiles per DMA group
SENT = 1000.0      # dst_rel sentinel for padding edges
W_S = D            # wsea columns: [0:160) exp*s
W_EA = D + H * ED  # [160:240) exp*ea
W_DEN = W_EA + H   # [240:245) exp ; padded to 248
W_PAD = 248


class Cfg:
    def __init__(self, n=N, e=E, ncores=NCORES):
        self.N, self.E, self.NCORES = n, e, ncores
        self.NV = n // ncores                      # nodes per core
        self.NBLK = math.ceil(self.NV / BLK)       # dst blocks per core
        self.NPAD = self.NBLK * BLK                # block-padded nodes
        self.NT = math.ceil(self.NPAD / P)         # node-phase tiles
        self.NTP = self.NT * P                     # 128-padded nodes
        self.G = None                              # set after edge prep


def _prep_edges(cfg, x, edge_index, edge_attr):
    """Sort/pad edges, stage the permuted x[src] and edge data per core."""
    src = np.asarray(edge_index[0]).astype(np.int64)
    dst = np.asarray(edge_index[1]).astype(np.int64)
    e = src.shape[0]
    core = dst // cfg.NV
    rel = dst - core * cfg.NV
    blk = rel // BLK
    lane = rel - blk * BLK
    gkey = core * cfg.NBLK + blk
    order = np.argsort(gkey, kind="stable")
    gcounts = np.bincount(gkey, minlength=cfg.NCORES * cfg.NBLK)
    counts = gcounts.reshape(cfg.NCORES, cfg.NBLK)
    chunks_per = -(-counts.max(axis=0) // P)       # [NBLK]
    chunks_per[chunks_per == 0] = 1
    S = int(chunks_per.sum())
    T = -(-S // CPT)
    G = -(-T // GROUP)
    T = G * GROUP
    S_pad = T * CPT
    cfg.G = G

    chunk_blk = np.full(S_pad, cfg.NBLK - 1, np.int64)
    chunk_base = np.zeros(cfg.NBLK, np.int64)
    pos = 0
    for b in range(cfg.NBLK):
        chunk_base[b] = pos
        nch = int(chunks_per[b])
        chunk_blk[pos:pos + nch] = b
        pos += nch
    first_chunk = np.zeros(cfg.NBLK, np.int64)
    last_chunk = np.zeros(cfg.NBLK, np.int64)
    for b in range(cfg.NBLK):
        w = np.nonzero(chunk_blk == b)[0]
        first_chunk[b], last_chunk[b] = w[0], w[-1]

    gstart = np.zeros_like(gcounts)
    gstart[1:] = np.cumsum(gcounts)[:-1]
    ranks = np.arange(e) - gstart[gkey[order]]
    ecore = core[order]
    eslot = chunk_base[blk[order]] * P + ranks       # slot in [0, S_pad*P)

    SL = S_pad * P
    xbf = np.asarray(x, np.float32).astype(BF)
    ea32 = np.asarray(edge_attr, np.float32).astype(BF)

    Xflat = np.zeros((cfg.NCORES, SL, D), BF)
    EAflat = np.zeros((cfg.NCORES, SL, ED), BF)
    drel_flat = np.full((cfg.NCORES, SL), SENT, np.float32)
    Xflat[ecore, eslot] = xbf[src[order]]
    EAflat[ecore, eslot] = ea32[order]
    drel_flat[ecore, eslot] = lane[order].astype(np.float32)

    # [NC, G, 8192, D] -> feature-major per group
    Xg = Xflat.reshape(cfg.NCORES, G, GROUP * CPT * P, D)
    xph = np.ascontiguousarray(Xg[:, :, :, 0:P].transpose(0, 1, 3, 2))
    xlo = np.zeros((cfg.NCORES, G, KLOD, GROUP * CPT * P), BF)
    xlo[:, :, 0:D - P, :] = Xg[:, :, :, P:D].transpose(0, 1, 3, 2)
    EAg = EAflat.reshape(cfg.NCORES, G, GROUP * CPT * P, ED)
    xlo[:, :, D - P:KLO, :] = EAg.transpose(0, 1, 3, 2)
    # edge-major ea rows: [NC, G, 128, GROUP*CPT*ED]
    ear = np.ascontiguousarray(
        EAg.reshape(cfg.NCORES, G, GROUP * CPT, P, ED).transpose(0, 1, 3, 2, 4)
        .reshape(cfg.NCORES, G, P, GROUP * CPT * ED))
    dg = drel_flat.reshape(cfg.NCORES, G, GROUP * CPT, P)
    drelT = np.ascontiguousarray(
        dg.transpose(0, 1, 3, 2)).astype(BF)       # [NC, G, 128, G*CPT]
    drow = np.ascontiguousarray(
        dg.reshape(cfg.NCORES, G, 1, GROUP * CPT * P)).astype(BF)

    sched = dict(
        T=T, G=G,
        chunk_blk=chunk_blk.tolist(),
        first_chunk=first_chunk.tolist(),
        last_chunk=last_chunk.tolist(),
    )
    return sched, xph, xlo, ear, drelT, drow


def _nontriv(a, v):
    return not np.all(np.asarray(a) == v)


def build_trace(cfg, sched, weights):
    G = sched["G"]
    chunk_blk = sched["chunk_blk"]
    first_chunk_of = {g: b for b, g in enumerate(sched["first_chunk"])}
    last_chunk_of = {g: b for b, g in enumerate(sched["last_chunk"])}

    W = weights
    use_bl = _nontriv(W["bl"], 0.0)
    use_br = _nontriv(W["br"], 0.0)
    use_bgat = _nontriv(W["b_gat"], 0.0)
    use_g1 = _nontriv(W["g1"], 1.0)
    use_b1 = _nontriv(W["b1"], 0.0)
    use_bm1 = _nontriv(W["b_m1"], 0.0)
    use_gm = _nontriv(W["g_m"], 1.0)
    use_bm = _nontriv(W["b_m"], 0.0)
    use_bm2 = _nontriv(W["b_m2"], 0.0)
    use_g2 = _nontriv(W["g2"], 1.0)
    use_b2 = _nontriv(W["b2"], 0.0)

    nc = bacc.Bacc("TRN2", target_bir_lowering=False, debug=False)

    NBLK, NT, NTP = cfg.NBLK, cfg.NT, cfg.NTP
    EPG = GROUP * CPT * P          # edges per group (8192)
    CPG = GROUP * CPT              # chunks per group (64)

    # ---------------- I/O declarations ----------------
    d_xph = nc.dram_tensor("xph", [G, P, EPG], BF16, kind="ExternalInput")
    d_xlo = nc.dram_tensor("xlo", [G, KLOD, EPG], BF16, kind="ExternalInput")
    d_ear = nc.dram_tensor("ear", [G, P, CPG * ED], BF16, kind="ExternalInput")
    d_drelT = nc.dram_tensor("drelT", [G, P, CPG], BF16, kind="ExternalInput")
    d_drow = nc.dram_tensor("drow", [G, 1, EPG], BF16, kind="ExternalInput")
    d_xoT_hi = nc.dram_tensor("xoT_hi", [P, NTP], BF16, kind="ExternalInput")
    d_xoT_lo = nc.dram_tensor("xoT_lo", [D - P, NTP], BF16, kind="ExternalInput")
    d_xown = nc.dram_tensor("x_own", [NTP, D], F32, kind="ExternalInput")
    d_out = nc.dram_tensor("y_out", [NTP, D], F32, kind="ExternalOutput")

    def inline(arr, name):
        return nc.inline_tensor(np.ascontiguousarray(arr), name=name)

    bf = lambda a: np.asarray(a, np.float32).astype(BF)
    Wl = np.asarray(W["Wl"], np.float32)
    We = np.asarray(W["We"], np.float32)
    c_Wl_hi = inline(bf(Wl[0:P, :]), "c_Wl_hi")
    # MM2 rhs top 48 rows: [Wl_lo; We]
    rhs_lo = np.concatenate([Wl[P:D, :], We], axis=0)       # [48, 160]
    c_rhs_lo = inline(bf(rhs_lo), "c_rhs_lo")
    c_Wr_hi = inline(bf(W["Wr"][0:P, :]), "c_Wr_hi")
    c_Wr_lo = inline(bf(W["Wr"][P:D, :]), "c_Wr_lo")
    # We5: block-diagonal per-head We  [80, 160]
    We5 = np.zeros((H * ED, D), np.float32)
    for h in range(H):
        We5[h * ED:(h + 1) * ED, h * C:(h + 1) * C] = We[:, h * C:(h + 1) * C]
    c_We5 = inline(bf(We5), "c_We5")
    c_ident = inline(np.eye(P, dtype=BF), "c_ident")
    c_att = inline(np.broadcast_to(
        bf(np.asarray(W["att"]).reshape(1, D)), (P, D)).copy(), "c_att")
    c_iota_c = inline(np.arange(BLK, dtype=np.float32).reshape(BLK, 1),
                      "c_iota_c")
    c_iota_r = inline(np.broadcast_to(
        np.arange(BLK, dtype=np.float32).reshape(1, BLK).astype(BF),
        (P, BLK)).copy(), "c_iota_r")
    c_Wm1_hi = inline(bf(W["W_m1"][0:P, :]), "c_Wm1_hi")
    c_Wm1_lo = inline(bf(W["W_m1"][P:D, :]), "c_Wm1_lo")
    c_Wm2 = inline(
        bf(W["W_m2"]).reshape(4, P, D).transpose(1, 0, 2).copy(), "c_Wm2")
    rows32 = lambda a: np.broadcast_to(
        np.asarray(a, np.float32).reshape(1, -1), (P, np.asarray(a).size)).copy()
    c_bl = inline(rows32(W["bl"]), "c_bl")
    c_br = inline(rows32(W["br"]), "c_br")
    c_bgat = inline(rows32(W["b_gat"]), "c_bgat")
    c_g1 = inline(rows32(W["g1"]), "c_g1")
    c_b1 = inline(rows32(W["b1"]), "c_b1")
    c_bm1 = inline(rows32(W["b_m1"]), "c_bm1")
    c_gm = inline(rows32(W["g_m"]), "c_gm")
    c_bm = inline(rows32(W["b_m"]), "c_bm")
    c_bm2 = inline(rows32(W["b_m2"]), "c_bm2")
    c_g2 = inline(rows32(W["g2"]), "c_g2")
    c_b2 = inline(rows32(W["b2"]), "c_b2")

    with tile.TileContext(nc) as tc:
        mesh_scope, _ = nc.enter_named_scope("ALGO_MESH", True)
        psp = tc.alloc_tile_pool(name="psp", bufs=8, space="PSUM")
        dram = tc.alloc_tile_pool(name="dram", bufs=1, space="DRAM")
        agg_d = dram.tile([NTP, W_PAD], F32, name="agg_d", tag="agg_d")
        xr_d = dram.tile([NTP, D], BF16, name="xr_d", tag="xr_d")

        cp = tc.alloc_tile_pool(name="consts", bufs=1)

        def csb(dr, shape, dtype, name):
            t = cp.tile(shape, dtype, name=name, tag=name)
            nc.sync.dma_start(out=t[tuple(slice(0, s) for s in shape)], in_=dr[:])
            return t

        Wl_hi = csb(c_Wl_hi, [P, D], BF16, "Wl_hi")
        rhs_lo_sb = csb(c_rhs_lo, [KLO, D], BF16, "rhs_lo_sb")
        Wr_hi = csb(c_Wr_hi, [P, D], BF16, "Wr_hi")
        Wr_lo = csb(c_Wr_lo, [D - P, D], BF16, "Wr_lo")
        We5_sb = csb(c_We5, [H * ED, D], BF16, "We5_sb")
        ident = csb(c_ident, [P, P], BF16, "ident")
        att_sb = csb(c_att, [P, D], BF16, "att_sb")
        iota_c = csb(c_iota_c, [BLK, 1], F32, "iota_c")
        iota_r = csb(c_iota_r, [P, BLK], BF16, "iota_r")
        Wm1_hi = csb(c_Wm1_hi, [P, HID], BF16, "Wm1_hi")
        Wm1_lo = csb(c_Wm1_lo, [D - P, HID], BF16, "Wm1_lo")
        Wm2_sb = csb(c_Wm2, [P, 4, D], BF16, "Wm2_sb")
        bl_sb = csb(c_bl, [P, D], F32, "bl_sb")
        br_sb = csb(c_br, [P, D], F32, "br_sb")
        bgat_sb = csb(c_bgat, [P, D], F32, "bgat_sb")
        g1_sb = csb(c_g1, [P, D], F32, "g1_sb")
        b1_sb = csb(c_b1, [P, D], F32, "b1_sb")
        bm1_sb = csb(c_bm1, [P, HID], F32, "bm1_sb")
        gm_sb = csb(c_gm, [P, HID], F32, "gm_sb")
        bm_sb = csb(c_bm, [P, HID], F32, "bm_sb")
        bm2_sb = csb(c_bm2, [P, D], F32, "bm2_sb")
        g2_sb = csb(c_g2, [P, D], F32, "g2_sb")
        b2_sb = csb(c_b2, [P, D], F32, "b2_sb")
        eps_sb = cp.tile([P, 1], F32, name="eps_sb", tag="eps_sb")
        nc.gpsimd.memset(eps_sb[:, :], float(EPS))
        lna_sb = cp.tile([P, 1], F32, name="lna_sb", tag="lna_sb")
        nc.gpsimd.memset(lna_sb[:, :], float(math.log(SELU_L * SELU_A)))

        # xoT kept resident: phase A (xr blocks) + phase C (xr recompute)
        xoT_hi = cp.tile([P, NTP], BF16, name="xoT_hi", tag="xoT_hi")
        nc.sync.dma_start(out=xoT_hi[:, :], in_=d_xoT_hi[:])
        xoT_lo = cp.tile([D - P, NTP], BF16, name="xoT_lo", tag="xoT_lo")
        nc.sync.dma_start(out=xoT_lo[:, :], in_=d_xoT_lo[:])

        # MM2 rhs per block: [Wl_lo; We; 0; xr_b]
        rhs_all = cp.tile([P, NBLK * D], BF16, name="rhs_all", tag="rhs_all")

        # ---------------- phase A: xr per block ----------------
        sc_xr, _ = nc.enter_named_scope("pA_xr", True)
        nc.gpsimd.memset(rhs_all[32:KLOD, :], 0.0)
        xr_sb = None
        for b in range(NBLK):
            nc.scalar.copy(out=rhs_all[0:KLO, b * D:(b + 1) * D],
                           in_=rhs_lo_sb[:, :])
            ps = psp.tile([BLK, D], F32, name=f"ps_xr{b}", tag="ps")
            cs = slice(b * BLK, (b + 1) * BLK)
            nc.tensor.matmul(ps[:, :], xoT_hi[:, cs], Wr_hi[:, :],
                             start=True, stop=False)
            nc.tensor.matmul(ps[:, :], xoT_lo[:, cs], Wr_lo[:, :],
                             start=False, stop=True)
            dst = rhs_all[KLOD:P, b * D:(b + 1) * D]
            if use_br:
                nc.vector.tensor_tensor(out=dst, in0=ps[:, :],
                                        in1=br_sb[0:BLK, :], op=AL.add)
            else:
                nc.vector.tensor_scalar_add(out=dst, in0=ps[:, :], scalar1=0.0)
            # stash xr rows for the node phase (incl. br if present)
            if xr_sb is None:
                xr_lo = b
                xr_sb = cp.tile([BLK, 8, D], BF16, name=f"xrsb{b}",
                                tag="xrsb", bufs=2)
            if use_br:
                nc.gpsimd.tensor_tensor(out=xr_sb[:, b - xr_lo, :],
                                        in0=ps[:, :], in1=br_sb[0:BLK, :],
                                        op=AL.add)
            else:
                nc.scalar.copy(out=xr_sb[:, b - xr_lo, :], in_=ps[:, :])
            if b - xr_lo == 7 or b == NBLK - 1:
                nb = b - xr_lo + 1
                nc.scalar.dma_start(
                    out=xr_d[xr_lo * BLK:(b + 1) * BLK, :]
                    .rearrange("(t l) w -> l t w", l=BLK),
                    in_=xr_sb[:, 0:nb, :])
                xr_sb = None
        nc.leave_named_scope("pA_xr", sc_xr, True)

        tc.strict_bb_all_engine_barrier()

        # ---------------- phase B: edges ----------------
        ep = tc.alloc_tile_pool(name="ep", bufs=1)
        sc_ed, _ = nc.enter_named_scope("pB_edge", True)
        # PE warm-up: sustained busy window pushes HAM to full clock
        prime = psp.tile([P, HID], F32, name="prime_b", tag="ps")
        for i in range(16):
            nc.tensor.matmul(prime[:, :], ident[:, :], Wm1_hi[:, :],
                             start=(i == 0), stop=(i == 15))
        agg_tiles = {}
        agg_sb = None
        agg_lo = 0
        for g in range(G):
            xph_t = ep.tile([P, EPG], BF16, name=f"xph{g}", tag="xph", bufs=2)
            comb_t = ep.tile([P, EPG], BF16, name=f"comb{g}", tag="comb", bufs=2)
            ear_t = ep.tile([P, CPG * ED], BF16, name=f"ear{g}", tag="ear",
                            bufs=2)
            drelT_t = ep.tile([P, CPG], BF16, name=f"drelT{g}", tag="drelT",
                              bufs=2)
            nc.sync.dma_start(out=xph_t[:, :], in_=d_xph[g])
            nc.sync.dma_start(out=comb_t[0:KLOD, :], in_=d_xlo[g])
            nc.sync.dma_start(out=ear_t[:, :], in_=d_ear[g])
            nc.sync.dma_start(out=drelT_t[:, :], in_=d_drelT[g])
            nc.sync.dma_start(out=comb_t[KLOD:P, :],
                              in_=d_drow[g].to_broadcast([BLK, EPG]))

            # one-hot masks: comb rows 64:128 (pick, in place), m2 (scatter)
            nc.vector.tensor_scalar(
                out=comb_t[KLOD:P, :], in0=comb_t[KLOD:P, :],
                scalar1=iota_c[:, 0:1], scalar2=None, op0=AL.is_equal)
            m2 = ep.tile([P, CPG, BLK], BF16, name=f"m2_{g}", tag="m2", bufs=2)
            nc.vector.tensor_tensor(
                out=m2[:, :, :],
                in0=iota_r[:, :].rearrange("p n -> p () n").to_broadcast(
                    [P, CPG, BLK]),
                in1=drelT_t[:, :].rearrange("p c -> p c ()").to_broadcast(
                    [P, CPG, BLK]),
                op=AL.is_equal)

            for t in range(GROUP):
                f_sb = ep.tile([P, CPT, D], BF16, name=f"f{g}_{t}", tag="f",
                               bufs=3)
                wsea = ep.tile([P, CPT, W_PAD], BF16, name=f"w{g}_{t}",
                               tag="w", bufs=3)
                sc_t = ep.tile([P, CPT, H], F32, name=f"sc{g}_{t}", tag="sc",
                               bufs=3)
                for k in range(CPT // 2):
                    ks = slice(2 * k, 2 * k + 2)
                    ps_s = psp.tile([P, 2, D], F32, name=f"ps_s{g}_{t}_{k}",
                                    tag="ps")
                    for j in range(2):
                        c = t * CPT + 2 * k + j
                        gi = g * CPG + c
                        b = chunk_blk[gi]
                        es = slice(c * P, (c + 1) * P)
                        nc.tensor.matmul(ps_s[:, j, :], xph_t[:, es],
                                         Wl_hi[:, :], start=True, stop=False)
                        nc.tensor.matmul(ps_s[:, j, :], comb_t[:, es],
                                         rhs_all[:, b * D:(b + 1) * D],
                                         start=False, stop=True)
                    # f = leaky_relu(s), fused into the PSUM read
                    nc.scalar.activation(out=f_sb[:, ks, :],
                                         in_=ps_s[:, :, :], func=AF.Prelu,
                                         alpha=float(SLOPE))
                    # f *= att (in place), per-head reduce, exp
                    nc.vector.tensor_tensor(
                        out=f_sb[:, ks, :], in0=f_sb[:, ks, :],
                        in1=att_sb[:, :].rearrange(
                            "p f -> p () f").to_broadcast([P, 2, D]),
                        op=AL.mult)
                    nc.vector.tensor_reduce(
                        out=sc_t[:, ks, :],
                        in_=f_sb[:, ks, :].rearrange(
                            "p c (h z) -> p c h z", z=C),
                        axis=mybir.AxisListType.X, op=AL.add)
                    nc.scalar.activation(out=wsea[:, ks, W_EA:W_DEN],
                                         in_=sc_t[:, ks, :], func=AF.Exp)
                    expv = wsea[:, ks, W_EA:W_DEN]
                    nc.vector.tensor_tensor(
                        out=wsea[:, ks, 0:D].rearrange(
                            "p c (h z) -> p c h z", z=C),
                        in0=ps_s[:, :, :].rearrange(
                            "p c (h z) -> p c h z", z=C),
                        in1=expv.rearrange(
                            "p c h -> p c h ()").to_broadcast([P, 2, H, C]),
                        op=AL.mult)
                    nc.vector.tensor_tensor(
                        out=wsea[:, ks, W_S:W_EA].rearrange(
                            "p c (h z) -> p c h z", z=ED),
                        in0=ear_t[:, :].rearrange(
                            "p (c z) -> p c () z",
                            z=ED)[:, t * CPT + 2 * k:t * CPT + 2 * k + 2]
                        .to_broadcast([P, 2, H, ED]),
                        in1=expv.rearrange("p c h -> p c h ()").to_broadcast(
                            [P, 2, H, ED]),
                        op=AL.mult)

                    for j in range(2):
                        c = t * CPT + 2 * k + j
                        gi = g * CPG + c
                        b = chunk_blk[gi]
                        if gi in first_chunk_of:
                            agg_tiles[b] = psp.tile([BLK, W_PAD], F32,
                                                    name=f"agg{b}", tag="ps")
                        at = agg_tiles[b]
                        nc.tensor.matmul(at[:, :], m2[:, c, :], wsea[:, c - t * CPT, :],
                                         start=(gi in first_chunk_of),
                                         stop=(gi in last_chunk_of))
                        if gi in last_chunk_of:
                            if agg_sb is None:
                                agg_lo = b
                                agg_sb = ep.tile([BLK, 8, W_PAD], F32,
                                                 name=f"aggsb{b}", tag="aggsb",
                                                 bufs=2)
                            nc.scalar.copy(out=agg_sb[:, b - agg_lo, :],
                                           in_=at[:, :])
                            del agg_tiles[b]
                            if b - agg_lo == 7 or b == NBLK - 1:
                                nb = b - agg_lo + 1
                                nc.scalar.dma_start(
                                    out=agg_d[agg_lo * BLK:(b + 1) * BLK, :]
                                    .rearrange("(t l) w -> l t w", l=BLK),
                                    in_=agg_sb[:, 0:nb, :])
                                agg_sb = None
        nc.leave_named_scope("pB_edge", sc_ed, True)

        tc.strict_bb_all_engine_barrier()
        ep.release()

        # ---------------- phase C: node phase ----------------
        npo = tc.alloc_tile_pool(name="np", bufs=1)
        sc_nd, _ = nc.enter_named_scope("pC_node", True)
        prime2 = psp.tile([P, HID], F32, name="prime_c", tag="ps")
        for i in range(16):
            nc.tensor.matmul(prime2[:, :], ident[:, :], Wm1_hi[:, :],
                             start=(i == 0), stop=(i == 15))
        NTG = 8
        for j0 in range(0, NT, NTG):
            nt = min(NTG, NT - j0)
            rs = slice(j0 * P, (j0 + nt) * P)
            x_g = npo.tile([P, NTG, D], F32, name=f"x{j0}", tag="x", bufs=2)
            a_g = npo.tile([P, NTG, W_PAD], F32, name=f"a{j0}", tag="a",
                           bufs=2)
            xr_g = npo.tile([P, NTG, D], BF16, name=f"xr{j0}", tag="xr",
                            bufs=2)
            nc.sync.dma_start(
                out=x_g[:, 0:nt, :],
                in_=d_xown[rs, :].rearrange("(t p) d -> p t d", p=P))
            nc.sync.dma_start(
                out=a_g[:, 0:nt, :],
                in_=agg_d[rs, :].rearrange("(t p) d -> p t d", p=P))
            nc.sync.dma_start(
                out=xr_g[:, 0:nt, :],
                in_=xr_d[rs, :].rearrange("(t p) d -> p t d", p=P))
            y_g = npo.tile([P, NTG, D], F32, name=f"y{j0}", tag="y", bufs=2)
            t0_g = npo.tile([P, NTG, D], F32, name=f"t0g{j0}", tag="t0g",
                            bufs=2)
            o1_g = npo.tile([P, NTG, D], F32, name=f"o1g{j0}", tag="o1g",
                            bufs=2)
            u2_g = npo.tile([P, NTG, HID], BF16, name=f"u2g{j0}", tag="u2g",
                            bufs=2)
            mv_g = npo.tile([P, 3, NTG, 3], F32, name=f"mvg{j0}", tag="mvg",
                            bufs=2)

            def ln_stats(src_ap, ph, jj, nm):
                st = npo.tile([P, 6], F32, name=f"st{nm}", tag="st", bufs=4)
                nc.vector.bn_stats(out=st[:, :], in_=src_ap)
                nc.vector.bn_aggr(out=mv_g[:, ph, jj, 0:2], in_=st[:, :])

            def ln_batch_rstd(ph):
                # rstd for all tiles of the group in one sqrt + one recip
                sd = npo.tile([P, NTG], F32, name=f"sd{j0}_{ph}", tag="sd",
                              bufs=4)
                nc.scalar.activation(out=sd[:, :], in_=mv_g[:, ph, :, 1],
                                     func=AF.Sqrt, bias=eps_sb[:, 0:1])
                nc.vector.reciprocal(out=mv_g[:, ph, :, 2], in_=sd[:, :])

            def ln_apply(src_ap, out_ap, width, ph, jj, g_sb, b_sb,
                         use_g, use_b):
                nc.vector.scalar_tensor_tensor(
                    out=out_ap, in0=src_ap, scalar=mv_g[:, ph, jj, 0:1],
                    in1=mv_g[:, ph, jj, 2:3].to_broadcast([P, width]),
                    op0=AL.subtract, op1=AL.mult)
                if use_g:
                    nc.vector.tensor_tensor(out=out_ap, in0=out_ap,
                                            in1=g_sb[:, :], op=AL.mult)
                if use_b:
                    nc.gpsimd.tensor_tensor(out=out_ap, in0=out_ap,
                                            in1=b_sb[:, :], op=AL.add)

            # ---- stage 1: corr, gating, t0, LN1 stats ----
            for jj in range(nt):
                j = j0 + jj
                x_t = x_g[:, jj, :]
                a_t = a_g[:, jj, :]
                t0 = t0_g[:, jj, :]

                sea_b = npo.tile([P, H * ED], BF16, name=f"seab{j}",
                                 tag="seab", bufs=3)
                nc.scalar.copy(out=sea_b[:, :], in_=a_t[:, W_S:W_EA])
                ps_t = psp.tile([H * ED, P], BF16, name=f"ps_t{j}", tag="ps")
                nc.tensor.transpose(out=ps_t[:, :], in_=sea_b[:, :],
                                    identity=ident[:, :])
                sea_T = npo.tile([H * ED, P], BF16, name=f"seaT{j}",
                                 tag="seaT", bufs=3)
                nc.scalar.copy(out=sea_T[:, :], in_=ps_t[:, :])
                ps_co = psp.tile([P, D], F32, name=f"ps_co{j}", tag="ps")
                nc.tensor.matmul(ps_co[:, :], sea_T[:, :], We5_sb[:, :],
                                 start=True, stop=True)

                den = npo.tile([P, H], F32, name=f"den{j}", tag="den", bufs=3)
                nc.vector.tensor_scalar_add(out=den[:, :],
                                            in0=a_t[:, W_EA:W_DEN],
                                            scalar1=1e-30)
                rec = npo.tile([P, 2, H], F32, name=f"rec{j}", tag="rec",
                               bufs=3)
                nc.vector.reciprocal(out=rec[:, 0, :], in_=den[:, :])
                nc.vector.tensor_tensor(out=rec[:, 1, :],
                                        in0=a_t[:, W_EA:W_DEN],
                                        in1=rec[:, 0, :], op=AL.mult)

                nc.vector.tensor_tensor(out=t0, in0=a_t[:, 0:D],
                                        in1=ps_co[:, :], op=AL.subtract)
                nc.vector.tensor_tensor(
                    out=t0.rearrange("p (h z) -> p h z", z=C),
                    in0=t0.rearrange("p (h z) -> p h z", z=C),
                    in1=rec[:, 0, :].rearrange("p h -> p h ()").to_broadcast(
                        [P, H, C]),
                    op=AL.mult)
                xrg = npo.tile([P, D], F32, name=f"xrg{j}", tag="xrg", bufs=3)
                nc.vector.tensor_tensor(
                    out=xrg[:, :].rearrange("p (h z) -> p h z", z=C),
                    in0=xr_g[:, jj, :].rearrange("p (h z) -> p h z", z=C),
                    in1=rec[:, 1, :].rearrange("p h -> p h ()").to_broadcast(
                        [P, H, C]),
                    op=AL.mult)
                nc.vector.tensor_tensor(out=t0, in0=t0, in1=xrg[:, :],
                                        op=AL.subtract)
                nc.vector.tensor_tensor(out=t0, in0=x_t, in1=t0, op=AL.add)
                if use_bl:
                    nc.gpsimd.tensor_tensor(out=t0, in0=t0,
                                            in1=bl_sb[:, :], op=AL.add)
                if use_bgat:
                    nc.gpsimd.tensor_tensor(out=t0, in0=t0,
                                            in1=bgat_sb[:, :], op=AL.add)
                ln_stats(t0, 0, jj, f"1_{j}")
            ln_batch_rstd(0)

            # ---- stage 2: LN1 apply, mm1, selu, LN2 stats ----
            for jj in range(nt):
                j = j0 + jj
                out1 = o1_g[:, jj, :]
                ln_apply(t0_g[:, jj, :], out1, D, 0, jj, g1_sb, b1_sb,
                         use_g1, use_b1)
                out1b = npo.tile([P, D], BF16, name=f"o1b{j}", tag="o1b",
                                 bufs=3)
                nc.scalar.copy(out=out1b[:, :], in_=out1)

                pt0 = psp.tile([P, P], BF16, name=f"pt0_{j}", tag="ps")
                nc.tensor.transpose(out=pt0[:, :], in_=out1b[:, 0:P],
                                    identity=ident[:, :])
                t0s = npo.tile([P, P], BF16, name=f"t0s{j}", tag="t0s", bufs=3)
                nc.scalar.copy(out=t0s[:, :], in_=pt0[:, :])
                pt1 = psp.tile([D - P, P], BF16, name=f"pt1_{j}", tag="ps")
                nc.tensor.transpose(out=pt1[:, :], in_=out1b[:, P:D],
                                    identity=ident[:, :])
                t1s = npo.tile([D - P, P], BF16, name=f"t1s{j}", tag="t1s",
                               bufs=3)
                nc.scalar.copy(out=t1s[:, :], in_=pt1[:, :])
                ps_h = psp.tile([P, HID], F32, name=f"ps_h{j}", tag="ps")
                nc.tensor.matmul(ps_h[:, :], t0s[:, :], Wm1_hi[:, :],
                                 start=True, stop=False)
                nc.tensor.matmul(ps_h[:, :], t1s[:, :], Wm1_lo[:, :],
                                 start=False, stop=True)

                if use_bm1:
                    y_sb = npo.tile([P, HID], F32, name=f"ysb{j}", tag="ysb",
                                    bufs=2)
                    nc.vector.tensor_tensor(out=y_sb[:, :], in0=ps_h[:, :],
                                            in1=bm1_sb[:, :], op=AL.add)
                    ysrc = y_sb[:, :]
                else:
                    ysrc = ps_h[:, :]
                e_sb = npo.tile([P, HID], BF16, name=f"esb{j}", tag="esb",
                                bufs=3)
                nc.scalar.activation(out=e_sb[:, :], in_=ysrc, func=AF.Exp,
                                     bias=lna_sb[:, 0:1])
                r_sb = npo.tile([P, HID], BF16, name=f"rsb{j}", tag="rsb",
                                bufs=3)
                nc.scalar.activation(out=r_sb[:, :], in_=ysrc, func=AF.Relu,
                                     scale=float(SELU_L))
                u2 = u2_g[:, jj, :]
                nc.vector.scalar_tensor_tensor(
                    out=u2, in0=e_sb[:, :], scalar=float(SELU_L * SELU_A),
                    in1=r_sb[:, :], op0=AL.min, op1=AL.add)
                ln_stats(u2, 1, jj, f"2_{j}")
            ln_batch_rstd(1)

            # ---- stage 3: LN2 apply, mm2, residual, LN3 stats ----
            for jj in range(nt):
                j = j0 + jj
                h_bf = npo.tile([P, HID], BF16, name=f"hbf{j}", tag="hbf",
                                bufs=3)
                ln_apply(u2_g[:, jj, :], h_bf[:, :], HID, 1, jj, gm_sb, bm_sb,
                         use_gm, use_bm)

                ps_m = psp.tile([P, D], F32, name=f"ps_m{j}", tag="ps")
                for k in range(4):
                    pth = psp.tile([P, P], BF16, name=f"pth{j}_{k}", tag="ps")
                    nc.tensor.transpose(out=pth[:, :],
                                        in_=h_bf[:, k * P:(k + 1) * P],
                                        identity=ident[:, :])
                    hts = npo.tile([P, P], BF16, name=f"hts{j}_{k}", tag="hts",
                                   bufs=4)
                    nc.scalar.copy(out=hts[:, :], in_=pth[:, :])
                    nc.tensor.matmul(ps_m[:, :], hts[:, :], Wm2_sb[:, k, :],
                                     start=(k == 0), stop=(k == 3))

                t2 = t0_g[:, jj, :]
                nc.vector.tensor_tensor(out=t2, in0=o1_g[:, jj, :],
                                        in1=ps_m[:, :], op=AL.add)
                if use_bm2:
                    nc.gpsimd.tensor_tensor(out=t2, in0=t2,
                                            in1=bm2_sb[:, :], op=AL.add)
                ln_stats(t2, 2, jj, f"3_{j}")
            ln_batch_rstd(2)

            # ---- stage 4: LN3 apply -> y ----
            for jj in range(nt):
                ln_apply(t0_g[:, jj, :], y_g[:, jj, :], D, 2, jj, g2_sb,
                         b2_sb, use_g2, use_b2)
            nc.scalar.dma_start(
                out=d_out[rs, :].rearrange("(t p) d -> p t d", p=P),
                in_=y_g[:, 0:nt, :])
        nc.leave_named_scope("pC_node", sc_nd, True)

        npo.release()
        psp.release()
        dram.release()
        cp.release()
        nc.leave_named_scope("ALGO_MESH", mesh_scope, True)

    nc.compile()
    return nc


def _make_in_maps(cfg, x, xph, xlo, ear, drelT, drow):
    x32 = np.asarray(x, np.float32)
    in_maps = []
    for k in range(cfg.NCORES):
        xo = np.zeros((cfg.NTP, D), np.float32)
        xo[:cfg.NV] = x32[k * cfg.NV:(k + 1) * cfg.NV]
        xoT = np.zeros((D, cfg.NTP), BF)
        xoT[:, :cfg.NV] = x32[k * cfg.NV:(k + 1) * cfg.NV].astype(BF).T
        in_maps.append({
            "xph": xph[k], "xlo": xlo[k], "ear": ear[k],
            "drelT": drelT[k], "drow": drow[k],
            "xoT_hi": np.ascontiguousarray(xoT[0:P]),
            "xoT_lo": np.ascontiguousarray(xoT[P:D]),
            "x_own": xo,
        })
    return in_maps


def build_all(inputs, cfg=None):
    cfg = cfg or Cfg()
    sched, xph, xlo, ear, drelT, drow = _prep_edges(
        cfg, inputs["x"], inputs["edge_index"], inputs["edge_attr"])
    wnames = ["Wl", "bl", "Wr", "br", "We", "att", "b_gat", "g1", "b1",
              "W_m1", "b_m1", "g_m", "b_m", "W_m2", "b_m2", "g2", "b2"]
    weights = {k: np.asarray(inputs[k], np.float32) for k in wnames}
    nc = build_trace(cfg, sched, weights)
    in_maps = _make_in_maps(cfg, inputs["x"], xph, xlo, ear, drelT, drow)
    return cfg, nc, in_maps


def kernel(**inputs) -> np.ndarray:
    cfg, nc, in_maps = build_all(inputs)
    res = run_bass_kernel_spmd(nc, in_maps, core_ids=list(range(cfg.NCORES)))
    out = np.concatenate(
        [r["y_out"][:cfg.NV] for r in res.results], axis=0
    ).astype(np.float32)
    return out


# revision 40
# speedup vs baseline: 1.5491x; 1.0018x over previous
"""Trainium2 Bass kernel for nn_NodeAttnModel (GATv2Conv + norm + MLP).

v2 architecture — no xl table, no dma_gather.

Key ideas:
  - Edges are sorted by destination and grouped into 80-node dst blocks;
    each 128-edge chunk belongs to one block.
  - The per-edge pre-activation  s = xl[src] + xr[dst] + ea@We  is produced
    by exactly TWO matmuls per chunk:
        MM1:  lhsT = Xp_hi (x[src].T rows 0:128, staged on host)  rhs = Wl_hi
        MM2:  lhsT = [Xp_lo(32); eaT(16); onehot_dst(80)]
              rhs  = [Wl_lo   ; We     ; xr_block     ]        (K = 128)
    The host supplies x[src] already permuted into edge order (it is a
    permutation of an *input*, so no on-device gather is needed).
  - Aggregation uses softmax linearity to avoid needing xl[src] per edge:
        agg = sum_e alpha*xl[src] = (S_s - S_ea@We5)/den - xr * den/(den+eps)
    where S_s = sum_e exp*s, S_ea = sum_e exp*ea (80 cols: 5 heads x 16),
    den = sum_e exp, all produced by ONE scatter matmul per chunk
    (rhs = [exp*s | exp*ea | exp], 245 cols, lhsT = dst one-hot).
  - Node phase: correction matmul + gating, then residual/LN/MLP as before.

All DMAs are large and batched (GROUP=8 tiles per call) to stay off the
descriptor-rate limits that dominated v1.
"""

import math

import numpy as np
import ml_dtypes

import concourse.bass as bass
import concourse.bacc as bacc
import concourse.mybir as mybir
import concourse.tile as tile
from concourse.bass_utils import run_bass_kernel_spmd

BF = ml_dtypes.bfloat16
F32 = mybir.dt.float32
BF16 = mybir.dt.bfloat16
AL = mybir.AluOpType
AF = mybir.ActivationFunctionType

# Problem constants
N, D, H, C, E, ED, HID = 50000, 160, 5, 32, 800000, 16, 512
EPS = 1e-5
SLOPE = 0.2
SELU_L = 1.0507009873554805
SELU_A = 1.6732632423543772

NCORES = 8
P = 128
BLK = 64           # dst nodes per block (mask rows 64:128 of the MM2 lhsT)
KLO = 48           # Xp_lo(32) + eaT(16) rows of the MM2 lhsT
KLOD = 64          # xlo DMA rows (48 data + 16 host zeros for K alignment)
CPT = 8            # chunks (of 128 edges) per tile
GROUP = 8          # tiles per DMA group
SENT = 1000.0      # dst_rel sentinel for padding edges
W_S = D            # wsea columns: [0:160) exp*s
W_EA = D + H * ED  # [160:240) exp*ea
W_DEN = W_EA + H   # [240:245) exp ; padded to 248
W_PAD = 248


class Cfg:
    def __init__(self, n=N, e=E, ncores=NCORES):
        self.N, self.E, self.NCORES = n, e, ncores
        self.NV = n // ncores                      # nodes per core
        self.NBLK = math.ceil(self.NV / BLK)       # dst blocks per core
        self.NPAD = self.NBLK * BLK                # block-padded nodes
        self.NT = math.ceil(self.NPAD / P)         # node-phase tiles
        self.NTP = self.NT * P                     # 128-padded nodes
        self.G = None                              # set after edge prep


def _prep_edges(cfg, x, edge_index, edge_attr):
    """Sort/pad edges, stage the permuted x[src] and edge data per core."""
    src = np.asarray(edge_index[0]).astype(np.int64)
    dst = np.asarray(edge_index[1]).astype(np.int64)
    e = src.shape[0]
    core = dst // cfg.NV
    rel = dst - core * cfg.NV
    blk = rel // BLK
    lane = rel - blk * BLK
    gkey = core * cfg.NBLK + blk
    order = np.argsort(gkey, kind="stable")
    gcounts = np.bincount(gkey, minlength=cfg.NCORES * cfg.NBLK)
    counts = gcounts.reshape(cfg.NCORES, cfg.NBLK)
    chunks_per = -(-counts.max(axis=0) // P)       # [NBLK]
    chunks_per[chunks_per == 0] = 1
    S = int(chunks_per.sum())
    T = -(-S // CPT)
    G = -(-T // GROUP)
    T = G * GROUP
    S_pad = T * CPT
    cfg.G = G

    chunk_blk = np.full(S_pad, cfg.NBLK - 1, np.int64)
    chunk_base = np.zeros(cfg.NBLK, np.int64)
    pos = 0
    for b in range(cfg.NBLK):
        chunk_base[b] = pos
        nch = int(chunks_per[b])
        chunk_blk[pos:pos + nch] = b
        pos += nch
    first_chunk = np.zeros(cfg.NBLK, np.int64)
    last_chunk = np.zeros(cfg.NBLK, np.int64)
    for b in range(cfg.NBLK):
        w = np.nonzero(chunk_blk == b)[0]
        first_chunk[b], last_chunk[b] = w[0], w[-1]

    gstart = np.zeros_like(gcounts)
    gstart[1:] = np.cumsum(gcounts)[:-1]
    ranks = np.arange(e) - gstart[gkey[order]]
    ecore = core[order]
    eslot = chunk_base[blk[order]] * P + ranks       # slot in [0, S_pad*P)

    SL = S_pad * P
    xbf = np.asarray(x, np.float32).astype(BF)
    ea32 = np.asarray(edge_attr, np.float32).astype(BF)

    Xflat = np.zeros((cfg.NCORES, SL, D), BF)
    EAflat = np.zeros((cfg.NCORES, SL, ED), BF)
    drel_flat = np.full((cfg.NCORES, SL), SENT, np.float32)
    Xflat[ecore, eslot] = xbf[src[order]]
    EAflat[ecore, eslot] = ea32[order]
    drel_flat[ecore, eslot] = lane[order].astype(np.float32)

    # [NC, G, 8192, D] -> feature-major per group
    Xg = Xflat.reshape(cfg.NCORES, G, GROUP * CPT * P, D)
    xph = np.ascontiguousarray(Xg[:, :, :, 0:P].transpose(0, 1, 3, 2))
    xlo = np.zeros((cfg.NCORES, G, KLOD, GROUP * CPT * P), BF)
    xlo[:, :, 0:D - P, :] = Xg[:, :, :, P:D].transpose(0, 1, 3, 2)
    EAg = EAflat.reshape(cfg.NCORES, G, GROUP * CPT * P, ED)
    xlo[:, :, D - P:KLO, :] = EAg.transpose(0, 1, 3, 2)
    # edge-major ea rows: [NC, G, 128, GROUP*CPT*ED]
    ear = np.ascontiguousarray(
        EAg.reshape(cfg.NCORES, G, GROUP * CPT, P, ED).transpose(0, 1, 3, 2, 4)
        .reshape(cfg.NCORES, G, P, GROUP * CPT * ED))
    dg = drel_flat.reshape(cfg.NCORES, G, GROUP * CPT, P)
    drelT = np.ascontiguousarray(
        dg.transpose(0, 1, 3, 2)).astype(BF)       # [NC, G, 128, G*CPT]
    drow = np.ascontiguousarray(
        dg.reshape(cfg.NCORES, G, 1, GROUP * CPT * P)).astype(BF)

    sched = dict(
        T=T, G=G,
        chunk_blk=chunk_blk.tolist(),
        first_chunk=first_chunk.tolist(),
        last_chunk=last_chunk.tolist(),
    )
    return sched, xph, xlo, ear, drelT, drow


def _nontriv(a, v):
    return not np.all(np.asarray(a) == v)


def build_trace(cfg, sched, weights):
    G = sched["G"]
    chunk_blk = sched["chunk_blk"]
    first_chunk_of = {g: b for b, g in enumerate(sched["first_chunk"])}
    last_chunk_of = {g: b for b, g in enumerate(sched["last_chunk"])}

    W = weights
    use_bl = _nontriv(W["bl"], 0.0)
    use_br = _nontriv(W["br"], 0.0)
    use_bgat = _nontriv(W["b_gat"], 0.0)
    use_g1 = _nontriv(W["g1"], 1.0)
    use_b1 = _nontriv(W["b1"], 0.0)
    use_bm1 = _nontriv(W["b_m1"], 0.0)
    use_gm = _nontriv(W["g_m"], 1.0)
    use_bm = _nontriv(W["b_m"], 0.0)
    use_bm2 = _nontriv(W["b_m2"], 0.0)
    use_g2 = _nontriv(W["g2"], 1.0)
    use_b2 = _nontriv(W["b2"], 0.0)

    nc = bacc.Bacc("TRN2", target_bir_lowering=False, debug=False)

    NBLK, NT, NTP = cfg.NBLK, cfg.NT, cfg.NTP
    EPG = GROUP * CPT * P          # edges per group (8192)
    CPG = GROUP * CPT              # chunks per group (64)

    # ---------------- I/O declarations ----------------
    d_xph = nc.dram_tensor("xph", [G, P, EPG], BF16, kind="ExternalInput")
    d_xlo = nc.dram_tensor("xlo", [G, KLOD, EPG], BF16, kind="ExternalInput")
    d_ear = nc.dram_tensor("ear", [G, P, CPG * ED], BF16, kind="ExternalInput")
    d_drelT = nc.dram_tensor("drelT", [G, P, CPG], BF16, kind="ExternalInput")
    d_drow = nc.dram_tensor("drow", [G, 1, EPG], BF16, kind="ExternalInput")
    d_xoT_hi = nc.dram_tensor("xoT_hi", [P, NTP], BF16, kind="ExternalInput")
    d_xoT_lo = nc.dram_tensor("xoT_lo", [D - P, NTP], BF16, kind="ExternalInput")
    d_xown = nc.dram_tensor("x_own", [NTP, D], F32, kind="ExternalInput")
    d_out = nc.dram_tensor("y_out", [NTP, D], F32, kind="ExternalOutput")

    def inline(arr, name):
        return nc.inline_tensor(np.ascontiguousarray(arr), name=name)

    bf = lambda a: np.asarray(a, np.float32).astype(BF)
    Wl = np.asarray(W["Wl"], np.float32)
    We = np.asarray(W["We"], np.float32)
    c_Wl_hi = inline(bf(Wl[0:P, :]), "c_Wl_hi")
    # MM2 rhs top 48 rows: [Wl_lo; We]
    rhs_lo = np.concatenate([Wl[P:D, :], We], axis=0)       # [48, 160]
    c_rhs_lo = inline(bf(rhs_lo), "c_rhs_lo")
    c_Wr_hi = inline(bf(W["Wr"][0:P, :]), "c_Wr_hi")
    c_Wr_lo = inline(bf(W["Wr"][P:D, :]), "c_Wr_lo")
    # We5: block-diagonal per-head We  [80, 160]
    We5 = np.zeros((H * ED, D), np.float32)
    for h in range(H):
        We5[h * ED:(h + 1) * ED, h * C:(h + 1) * C] = We[:, h * C:(h + 1) * C]
    c_We5 = inline(bf(We5), "c_We5")
    c_ident = inline(np.eye(P, dtype=BF), "c_ident")
    c_att = inline(np.broadcast_to(
        bf(np.asarray(W["att"]).reshape(1, D)), (P, D)).copy(), "c_att")
    c_iota_c = inline(np.arange(BLK, dtype=np.float32).reshape(BLK, 1),
                      "c_iota_c")
    c_iota_r = inline(np.broadcast_to(
        np.arange(BLK, dtype=np.float32).reshape(1, BLK).astype(BF),
        (P, BLK)).copy(), "c_iota_r")
    c_Wm1_hi = inline(bf(W["W_m1"][0:P, :]), "c_Wm1_hi")
    c_Wm1_lo = inline(bf(W["W_m1"][P:D, :]), "c_Wm1_lo")
    c_Wm2 = inline(
        bf(W["W_m2"]).reshape(4, P, D).transpose(1, 0, 2).copy(), "c_Wm2")
    rows32 = lambda a: np.broadcast_to(
        np.asarray(a, np.float32).reshape(1, -1), (P, np.asarray(a).size)).copy()
    c_bl = inline(rows32(W["bl"]), "c_bl")
    c_br = inline(rows32(W["br"]), "c_br")
    c_bgat = inline(rows32(W["b_gat"]), "c_bgat")
    c_g1 = inline(rows32(W["g1"]), "c_g1")
    c_b1 = inline(rows32(W["b1"]), "c_b1")
    c_bm1 = inline(rows32(W["b_m1"]), "c_bm1")
    c_gm = inline(rows32(W["g_m"]), "c_gm")
    c_bm = inline(rows32(W["b_m"]), "c_bm")
    c_bm2 = inline(rows32(W["b_m2"]), "c_bm2")
    c_g2 = inline(rows32(W["g2"]), "c_g2")
    c_b2 = inline(rows32(W["b2"]), "c_b2")

    with tile.TileContext(nc) as tc:
        mesh_scope, _ = nc.enter_named_scope("ALGO_MESH", True)
        psp = tc.alloc_tile_pool(name="psp", bufs=8, space="PSUM")
        dram = tc.alloc_tile_pool(name="dram", bufs=1, space="DRAM")
        agg_d = dram.tile([NTP, W_PAD], F32, name="agg_d", tag="agg_d")
        xr_d = dram.tile([NTP, D], BF16, name="xr_d", tag="xr_d")

        cp = tc.alloc_tile_pool(name="consts", bufs=1)

        def csb(dr, shape, dtype, name):
            t = cp.tile(shape, dtype, name=name, tag=name)
            nc.sync.dma_start(out=t[tuple(slice(0, s) for s in shape)], in_=dr[:])
            return t

        Wl_hi = csb(c_Wl_hi, [P, D], BF16, "Wl_hi")
        rhs_lo_sb = csb(c_rhs_lo, [KLO, D], BF16, "rhs_lo_sb")
        Wr_hi = csb(c_Wr_hi, [P, D], BF16, "Wr_hi")
        Wr_lo = csb(c_Wr_lo, [D - P, D], BF16, "Wr_lo")
        We5_sb = csb(c_We5, [H * ED, D], BF16, "We5_sb")
        ident = csb(c_ident, [P, P], BF16, "ident")
        att_sb = csb(c_att, [P, D], BF16, "att_sb")
        iota_c = csb(c_iota_c, [BLK, 1], F32, "iota_c")
        iota_r = csb(c_iota_r, [P, BLK], BF16, "iota_r")
        Wm1_hi = csb(c_Wm1_hi, [P, HID], BF16, "Wm1_hi")
        Wm1_lo = csb(c_Wm1_lo, [D - P, HID], BF16, "Wm1_lo")
        Wm2_sb = csb(c_Wm2, [P, 4, D], BF16, "Wm2_sb")
        bl_sb = csb(c_bl, [P, D], F32, "bl_sb")
        br_sb = csb(c_br, [P, D], F32, "br_sb")
        bgat_sb = csb(c_bgat, [P, D], F32, "bgat_sb")
        g1_sb = csb(c_g1, [P, D], F32, "g1_sb")
        b1_sb = csb(c_b1, [P, D], F32, "b1_sb")
        bm1_sb = csb(c_bm1, [P, HID], F32, "bm1_sb")
        gm_sb = csb(c_gm, [P, HID], F32, "gm_sb")
        bm_sb = csb(c_bm, [P, HID], F32, "bm_sb")
        bm2_sb = csb(c_bm2, [P, D], F32, "bm2_sb")
        g2_sb = csb(c_g2, [P, D], F32, "g2_sb")
        b2_sb = csb(c_b2, [P, D], F32, "b2_sb")
        eps_sb = cp.tile([P, 1], F32, name="eps_sb", tag="eps_sb")
        nc.gpsimd.memset(eps_sb[:, :], float(EPS))
        lna_sb = cp.tile([P, 1], F32, name="lna_sb", tag="lna_sb")
        nc.gpsimd.memset(lna_sb[:, :], float(math.log(SELU_L * SELU_A)))

        # xoT kept resident: phase A (xr blocks) + phase C (xr recompute)
        xoT_hi = cp.tile([P, NTP], BF16, name="xoT_hi", tag="xoT_hi")
        nc.sync.dma_start(out=xoT_hi[:, :], in_=d_xoT_hi[:])
        xoT_lo = cp.tile([D - P, NTP], BF16, name="xoT_lo", tag="xoT_lo")
        nc.sync.dma_start(out=xoT_lo[:, :], in_=d_xoT_lo[:])

        # MM2 rhs per block: [Wl_lo; We; 0; xr_b]
        rhs_all = cp.tile([P, NBLK * D], BF16, name="rhs_all", tag="rhs_all")

        # ---------------- phase A: xr per block ----------------
        sc_xr, _ = nc.enter_named_scope("pA_xr", True)
        nc.gpsimd.memset(rhs_all[32:KLOD, :], 0.0)
        xr_sb = None
        for b in range(NBLK):
            nc.scalar.copy(out=rhs_all[0:KLO, b * D:(b + 1) * D],
                           in_=rhs_lo_sb[:, :])
            ps = psp.tile([BLK, D], F32, name=f"ps_xr{b}", tag="ps")
            cs = slice(b * BLK, (b + 1) * BLK)
            nc.tensor.matmul(ps[:, :], xoT_hi[:, cs], Wr_hi[:, :],
                             start=True, stop=False)
            nc.tensor.matmul(ps[:, :], xoT_lo[:, cs], Wr_lo[:, :],
                             start=False, stop=True)
            dst = rhs_all[KLOD:P, b * D:(b + 1) * D]
            if use_br:
                nc.vector.tensor_tensor(out=dst, in0=ps[:, :],
                                        in1=br_sb[0:BLK, :], op=AL.add)
            else:
                nc.vector.tensor_scalar_add(out=dst, in0=ps[:, :], scalar1=0.0)
            # stash xr rows for the node phase (incl. br if present)
            if xr_sb is None:
                xr_lo = b
                xr_sb = cp.tile([BLK, 8, D], BF16, name=f"xrsb{b}",
                                tag="xrsb", bufs=2)
            if use_br:
                nc.gpsimd.tensor_tensor(out=xr_sb[:, b - xr_lo, :],
                                        in0=ps[:, :], in1=br_sb[0:BLK, :],
                                        op=AL.add)
            else:
                nc.scalar.copy(out=xr_sb[:, b - xr_lo, :], in_=ps[:, :])
            if b - xr_lo == 7 or b == NBLK - 1:
                nb = b - xr_lo + 1
                nc.scalar.dma_start(
                    out=xr_d[xr_lo * BLK:(b + 1) * BLK, :]
                    .rearrange("(t l) w -> l t w", l=BLK),
                    in_=xr_sb[:, 0:nb, :])
                xr_sb = None
        nc.leave_named_scope("pA_xr", sc_xr, True)

        tc.strict_bb_all_engine_barrier()

        # ---------------- phase B: edges ----------------
        ep = tc.alloc_tile_pool(name="ep", bufs=1)
        sc_ed, _ = nc.enter_named_scope("pB_edge", True)
        # PE warm-up: sustained busy window pushes HAM to full clock
        prime = psp.tile([P, HID], F32, name="prime_b", tag="ps")
        for i in range(16):
            nc.tensor.matmul(prime[:, :], ident[:, :], Wm1_hi[:, :],
                             start=(i == 0), stop=(i == 15))
        agg_tiles = {}
        agg_sb = None
        agg_lo = 0
        for g in range(G):
            xph_t = ep.tile([P, EPG], BF16, name=f"xph{g}", tag="xph", bufs=2)
            comb_t = ep.tile([P, EPG], BF16, name=f"comb{g}", tag="comb", bufs=2)
            ear_t = ep.tile([P, CPG * ED], BF16, name=f"ear{g}", tag="ear",
                            bufs=2)
            drelT_t = ep.tile([P, CPG], BF16, name=f"drelT{g}", tag="drelT",
                              bufs=2)
            nc.sync.dma_start(out=xph_t[:, :], in_=d_xph[g])
            nc.sync.dma_start(out=comb_t[0:KLOD, :], in_=d_xlo[g])
            nc.sync.dma_start(out=ear_t[:, :], in_=d_ear[g])
            nc.sync.dma_start(out=drelT_t[:, :], in_=d_drelT[g])
            nc.sync.dma_start(out=comb_t[KLOD:P, :],
                              in_=d_drow[g].to_broadcast([BLK, EPG]))

            # one-hot masks: comb rows 64:128 (pick, in place), m2 (scatter)
            nc.vector.tensor_scalar(
                out=comb_t[KLOD:P, :], in0=comb_t[KLOD:P, :],
                scalar1=iota_c[:, 0:1], scalar2=None, op0=AL.is_equal)
            m2 = ep.tile([P, CPG, BLK], BF16, name=f"m2_{g}", tag="m2", bufs=2)
            nc.vector.tensor_tensor(
                out=m2[:, :, :],
                in0=iota_r[:, :].rearrange("p n -> p () n").to_broadcast(
                    [P, CPG, BLK]),
                in1=drelT_t[:, :].rearrange("p c -> p c ()").to_broadcast(
                    [P, CPG, BLK]),
                op=AL.is_equal)

            for t in range(GROUP):
                f_sb = ep.tile([P, CPT, D], BF16, name=f"f{g}_{t}", tag="f",
                               bufs=3)
                wsea = ep.tile([P, CPT, W_PAD], BF16, name=f"w{g}_{t}",
                               tag="w", bufs=3)
                sc_t = ep.tile([P, CPT, H], BF16, name=f"sc{g}_{t}", tag="sc",
                               bufs=3)
                for k in range(CPT // 2):
                    ks = slice(2 * k, 2 * k + 2)
                    ps_s = psp.tile([P, 2, D], F32, name=f"ps_s{g}_{t}_{k}",
                                    tag="ps")
                    for j in range(2):
                        c = t * CPT + 2 * k + j
                        gi = g * CPG + c
                        b = chunk_blk[gi]
                        es = slice(c * P, (c + 1) * P)
                        nc.tensor.matmul(ps_s[:, j, :], xph_t[:, es],
                                         Wl_hi[:, :], start=True, stop=False)
                        nc.tensor.matmul(ps_s[:, j, :], comb_t[:, es],
                                         rhs_all[:, b * D:(b + 1) * D],
                                         start=False, stop=True)
                    # f = leaky_relu(s), fused into the PSUM read
                    nc.scalar.activation(out=f_sb[:, ks, :],
                                         in_=ps_s[:, :, :], func=AF.Prelu,
                                         alpha=float(SLOPE))
                    # f *= att (in place), per-head reduce, exp
                    nc.vector.tensor_tensor(
                        out=f_sb[:, ks, :], in0=f_sb[:, ks, :],
                        in1=att_sb[:, :].rearrange(
                            "p f -> p () f").to_broadcast([P, 2, D]),
                        op=AL.mult)
                    with nc.allow_low_precision(
                            reason="bf16 score reduce, |score|<~3"):
                        nc.vector.tensor_reduce(
                            out=sc_t[:, ks, :],
                            in_=f_sb[:, ks, :].rearrange(
                                "p c (h z) -> p c h z", z=C),
                            axis=mybir.AxisListType.X, op=AL.add)
                    nc.scalar.activation(out=wsea[:, ks, W_EA:W_DEN],
                                         in_=sc_t[:, ks, :], func=AF.Exp)
                    expv = wsea[:, ks, W_EA:W_DEN]
                    nc.vector.tensor_tensor(
                        out=wsea[:, ks, 0:D].rearrange(
                            "p c (h z) -> p c h z", z=C),
                        in0=ps_s[:, :, :].rearrange(
                            "p c (h z) -> p c h z", z=C),
                        in1=expv.rearrange(
                            "p c h -> p c h ()").to_broadcast([P, 2, H, C]),
                        op=AL.mult)
                    nc.vector.tensor_tensor(
                        out=wsea[:, ks, W_S:W_EA].rearrange(
                            "p c (h z) -> p c h z", z=ED),
                        in0=ear_t[:, :].rearrange(
                            "p (c z) -> p c () z",
                            z=ED)[:, t * CPT + 2 * k:t * CPT + 2 * k + 2]
                        .to_broadcast([P, 2, H, ED]),
                        in1=expv.rearrange("p c h -> p c h ()").to_broadcast(
                            [P, 2, H, ED]),
                        op=AL.mult)

                    for j in range(2):
                        c = t * CPT + 2 * k + j
                        gi = g * CPG + c
                        b = chunk_blk[gi]
                        if gi in first_chunk_of:
                            agg_tiles[b] = psp.tile([BLK, W_PAD], F32,
                                                    name=f"agg{b}", tag="ps")
                        at = agg_tiles[b]
                        nc.tensor.matmul(at[:, :], m2[:, c, :], wsea[:, c - t * CPT, :],
                                         start=(gi in first_chunk_of),
                                         stop=(gi in last_chunk_of))
                        if gi in last_chunk_of:
                            if agg_sb is None:
                                agg_lo = b
                                agg_sb = ep.tile([BLK, 8, W_PAD], F32,
                                                 name=f"aggsb{b}", tag="aggsb",
                                                 bufs=2)
                            nc.scalar.copy(out=agg_sb[:, b - agg_lo, :],
                                           in_=at[:, :])
                            del agg_tiles[b]
                            if b - agg_lo == 7 or b == NBLK - 1:
                                nb = b - agg_lo + 1
                                nc.scalar.dma_start(
                                    out=agg_d[agg_lo * BLK:(b + 1) * BLK, :]
                                    .rearrange("(t l) w -> l t w", l=BLK),
                                    in_=agg_sb[:, 0:nb, :])
                                agg_sb = None
        nc.leave_named_scope("pB_edge", sc_ed, True)

        tc.strict_bb_all_engine_barrier()
        ep.release()

        # ---------------- phase C: node phase ----------------
        npo = tc.alloc_tile_pool(name="np", bufs=1)
        sc_nd, _ = nc.enter_named_scope("pC_node", True)
        prime2 = psp.tile([P, HID], F32, name="prime_c", tag="ps")
        for i in range(16):
            nc.tensor.matmul(prime2[:, :], ident[:, :], Wm1_hi[:, :],
                             start=(i == 0), stop=(i == 15))
        NTG = 8
        for j0 in range(0, NT, NTG):
            nt = min(NTG, NT - j0)
            rs = slice(j0 * P, (j0 + nt) * P)
            x_g = npo.tile([P, NTG, D], F32, name=f"x{j0}", tag="x", bufs=2)
            a_g = npo.tile([P, NTG, W_PAD], F32, name=f"a{j0}", tag="a",
                           bufs=2)
            xr_g = npo.tile([P, NTG, D], BF16, name=f"xr{j0}", tag="xr",
                            bufs=2)
            nc.sync.dma_start(
                out=x_g[:, 0:nt, :],
                in_=d_xown[rs, :].rearrange("(t p) d -> p t d", p=P))
            nc.sync.dma_start(
                out=a_g[:, 0:nt, :],
                in_=agg_d[rs, :].rearrange("(t p) d -> p t d", p=P))
            nc.sync.dma_start(
                out=xr_g[:, 0:nt, :],
                in_=xr_d[rs, :].rearrange("(t p) d -> p t d", p=P))
            y_g = npo.tile([P, NTG, D], F32, name=f"y{j0}", tag="y", bufs=2)
            t0_g = npo.tile([P, NTG, D], F32, name=f"t0g{j0}", tag="t0g",
                            bufs=2)
            o1_g = npo.tile([P, NTG, D], F32, name=f"o1g{j0}", tag="o1g",
                            bufs=2)
            u2_g = npo.tile([P, NTG, HID], BF16, name=f"u2g{j0}", tag="u2g",
                            bufs=2)
            mv_g = npo.tile([P, 3, NTG, 3], F32, name=f"mvg{j0}", tag="mvg",
                            bufs=2)

            def ln_stats(src_ap, ph, jj, nm):
                st = npo.tile([P, 6], F32, name=f"st{nm}", tag="st", bufs=4)
                nc.vector.bn_stats(out=st[:, :], in_=src_ap)
                nc.vector.bn_aggr(out=mv_g[:, ph, jj, 0:2], in_=st[:, :])

            def ln_batch_rstd(ph):
                # rstd for all tiles of the group in one sqrt + one recip
                sd = npo.tile([P, NTG], F32, name=f"sd{j0}_{ph}", tag="sd",
                              bufs=4)
                nc.scalar.activation(out=sd[:, :], in_=mv_g[:, ph, :, 1],
                                     func=AF.Sqrt, bias=eps_sb[:, 0:1])
                nc.vector.reciprocal(out=mv_g[:, ph, :, 2], in_=sd[:, :])

            def ln_apply(src_ap, out_ap, width, ph, jj, g_sb, b_sb,
                         use_g, use_b):
                nc.vector.scalar_tensor_tensor(
                    out=out_ap, in0=src_ap, scalar=mv_g[:, ph, jj, 0:1],
                    in1=mv_g[:, ph, jj, 2:3].to_broadcast([P, width]),
                    op0=AL.subtract, op1=AL.mult)
                if use_g:
                    nc.vector.tensor_tensor(out=out_ap, in0=out_ap,
                                            in1=g_sb[:, :], op=AL.mult)
                if use_b:
                    nc.gpsimd.tensor_tensor(out=out_ap, in0=out_ap,
                                            in1=b_sb[:, :], op=AL.add)

            # ---- stage 1: corr, gating, t0, LN1 stats ----
            for jj in range(nt):
                j = j0 + jj
                x_t = x_g[:, jj, :]
                a_t = a_g[:, jj, :]
                t0 = t0_g[:, jj, :]

                sea_b = npo.tile([P, H * ED], BF16, name=f"seab{j}",
                                 tag="seab", bufs=3)
                nc.scalar.copy(out=sea_b[:, :], in_=a_t[:, W_S:W_EA])
                ps_t = psp.tile([H * ED, P], BF16, name=f"ps_t{j}", tag="ps")
                nc.tensor.transpose(out=ps_t[:, :], in_=sea_b[:, :],
                                    identity=ident[:, :])
                sea_T = npo.tile([H * ED, P], BF16, name=f"seaT{j}",
                                 tag="seaT", bufs=3)
                nc.scalar.copy(out=sea_T[:, :], in_=ps_t[:, :])
                ps_co = psp.tile([P, D], F32, name=f"ps_co{j}", tag="ps")
                nc.tensor.matmul(ps_co[:, :], sea_T[:, :], We5_sb[:, :],
                                 start=True, stop=True)

                den = npo.tile([P, H], F32, name=f"den{j}", tag="den", bufs=3)
                nc.vector.tensor_scalar_add(out=den[:, :],
                                            in0=a_t[:, W_EA:W_DEN],
                                            scalar1=1e-30)
                rec = npo.tile([P, 2, H], F32, name=f"rec{j}", tag="rec",
                               bufs=3)
                nc.vector.reciprocal(out=rec[:, 0, :], in_=den[:, :])
                nc.vector.tensor_tensor(out=rec[:, 1, :],
                                        in0=a_t[:, W_EA:W_DEN],
                                        in1=rec[:, 0, :], op=AL.mult)

                nc.vector.tensor_tensor(out=t0, in0=a_t[:, 0:D],
                                        in1=ps_co[:, :], op=AL.subtract)
                nc.vector.tensor_tensor(
                    out=t0.rearrange("p (h z) -> p h z", z=C),
                    in0=t0.rearrange("p (h z) -> p h z", z=C),
                    in1=rec[:, 0, :].rearrange("p h -> p h ()").to_broadcast(
                        [P, H, C]),
                    op=AL.mult)
                xrg = npo.tile([P, D], F32, name=f"xrg{j}", tag="xrg", bufs=3)
                nc.vector.tensor_tensor(
                    out=xrg[:, :].rearrange("p (h z) -> p h z", z=C),
                    in0=xr_g[:, jj, :].rearrange("p (h z) -> p h z", z=C),
                    in1=rec[:, 1, :].rearrange("p h -> p h ()").to_broadcast(
                        [P, H, C]),
                    op=AL.mult)
                nc.vector.tensor_tensor(out=t0, in0=t0, in1=xrg[:, :],
                                        op=AL.subtract)
                nc.vector.tensor_tensor(out=t0, in0=x_t, in1=t0, op=AL.add)
                if use_bl:
                    nc.gpsimd.tensor_tensor(out=t0, in0=t0,
                                            in1=bl_sb[:, :], op=AL.add)
                if use_bgat:
                    nc.gpsimd.tensor_tensor(out=t0, in0=t0,
                                            in1=bgat_sb[:, :], op=AL.add)
                ln_stats(t0, 0, jj, f"1_{j}")
            ln_batch_rstd(0)

            # ---- stage 2: LN1 apply, mm1, selu, LN2 stats ----
            for jj in range(nt):
                j = j0 + jj
                out1 = o1_g[:, jj, :]
                ln_apply(t0_g[:, jj, :], out1, D, 0, jj, g1_sb, b1_sb,
                         use_g1, use_b1)
                out1b = npo.tile([P, D], BF16, name=f"o1b{j}", tag="o1b",
                                 bufs=3)
                nc.scalar.copy(out=out1b[:, :], in_=out1)

                pt0 = psp.tile([P, P], BF16, name=f"pt0_{j}", tag="ps")
                nc.tensor.transpose(out=pt0[:, :], in_=out1b[:, 0:P],
                                    identity=ident[:, :])
                t0s = npo.tile([P, P], BF16, name=f"t0s{j}", tag="t0s", bufs=3)
                nc.scalar.copy(out=t0s[:, :], in_=pt0[:, :])
                pt1 = psp.tile([D - P, P], BF16, name=f"pt1_{j}", tag="ps")
                nc.tensor.transpose(out=pt1[:, :], in_=out1b[:, P:D],
                                    identity=ident[:, :])
                t1s = npo.tile([D - P, P], BF16, name=f"t1s{j}", tag="t1s",
                               bufs=3)
                nc.scalar.copy(out=t1s[:, :], in_=pt1[:, :])
                ps_h = psp.tile([P, HID], F32, name=f"ps_h{j}", tag="ps")
                nc.tensor.matmul(ps_h[:, :], t0s[:, :], Wm1_hi[:, :],
                                 start=True, stop=False)
                nc.tensor.matmul(ps_h[:, :], t1s[:, :], Wm1_lo[:, :],
                                 start=False, stop=True)

                if use_bm1:
                    y_sb = npo.tile([P, HID], F32, name=f"ysb{j}", tag="ysb",
                                    bufs=2)
                    nc.vector.tensor_tensor(out=y_sb[:, :], in0=ps_h[:, :],
                                            in1=bm1_sb[:, :], op=AL.add)
                    ysrc = y_sb[:, :]
                else:
                    ysrc = ps_h[:, :]
                e_sb = npo.tile([P, HID], BF16, name=f"esb{j}", tag="esb",
                                bufs=3)
                nc.scalar.activation(out=e_sb[:, :], in_=ysrc, func=AF.Exp,
                                     bias=lna_sb[:, 0:1])
                r_sb = npo.tile([P, HID], BF16, name=f"rsb{j}", tag="rsb",
                                bufs=3)
                nc.scalar.activation(out=r_sb[:, :], in_=ysrc, func=AF.Relu,
                                     scale=float(SELU_L))
                u2 = u2_g[:, jj, :]
                nc.vector.scalar_tensor_tensor(
                    out=u2, in0=e_sb[:, :], scalar=float(SELU_L * SELU_A),
                    in1=r_sb[:, :], op0=AL.min, op1=AL.add)
                ln_stats(u2, 1, jj, f"2_{j}")
            ln_batch_rstd(1)

            # ---- stage 3: LN2 apply, mm2, residual, LN3 stats ----
            for jj in range(nt):
                j = j0 + jj
                h_bf = npo.tile([P, HID], BF16, name=f"hbf{j}", tag="hbf",
                                bufs=3)
                ln_apply(u2_g[:, jj, :], h_bf[:, :], HID, 1, jj, gm_sb, bm_sb,
                         use_gm, use_bm)

                ps_m = psp.tile([P, D], F32, name=f"ps_m{j}", tag="ps")
                for k in range(4):
                    pth = psp.tile([P, P], BF16, name=f"pth{j}_{k}", tag="ps")
                    nc.tensor.transpose(out=pth[:, :],
                                        in_=h_bf[:, k * P:(k + 1) * P],
                                        identity=ident[:, :])
                    hts = npo.tile([P, P], BF16, name=f"hts{j}_{k}", tag="hts",
                                   bufs=4)
                    nc.scalar.copy(out=hts[:, :], in_=pth[:, :])
                    nc.tensor.matmul(ps_m[:, :], hts[:, :], Wm2_sb[:, k, :],
                                     start=(k == 0), stop=(k == 3))

                t2 = t0_g[:, jj, :]
                nc.vector.tensor_tensor(out=t2, in0=o1_g[:, jj, :],
                                        in1=ps_m[:, :], op=AL.add)
                if use_bm2:
                    nc.gpsimd.tensor_tensor(out=t2, in0=t2,
                                            in1=bm2_sb[:, :], op=AL.add)
                ln_stats(t2, 2, jj, f"3_{j}")
            ln_batch_rstd(2)

            # ---- stage 4: LN3 apply -> y ----
            for jj in range(nt):
                ln_apply(t0_g[:, jj, :], y_g[:, jj, :], D, 2, jj, g2_sb,
                         b2_sb, use_g2, use_b2)
            nc.scalar.dma_start(
                out=d_out[rs, :].rearrange("(t p) d -> p t d", p=P),
                in_=y_g[:, 0:nt, :])
        nc.leave_named_scope("pC_node", sc_nd, True)

        npo.release()
        psp.release()
        dram.release()
        cp.release()
        nc.leave_named_scope("ALGO_MESH", mesh_scope, True)

    nc.compile()
    return nc


def _make_in_maps(cfg, x, xph, xlo, ear, drelT, drow):
    x32 = np.asarray(x, np.float32)
    in_maps = []
    for k in range(cfg.NCORES):
        xo = np.zeros((cfg.NTP, D), np.float32)
        xo[:cfg.NV] = x32[k * cfg.NV:(k + 1) * cfg.NV]
        xoT = np.zeros((D, cfg.NTP), BF)
        xoT[:, :cfg.NV] = x32[k * cfg.NV:(k + 1) * cfg.NV].astype(BF).T
        in_maps.append({
            "xph": xph[k], "xlo": xlo[k], "ear": ear[k],
            "drelT": drelT[k], "drow": drow[k],
            "xoT_hi": np.ascontiguousarray(xoT[0:P]),
            "xoT_lo": np.ascontiguousarray(xoT[P:D]),
            "x_own": xo,
        })
    return in_maps


def build_all(inputs, cfg=None):
    cfg = cfg or Cfg()
    sched, xph, xlo, ear, drelT, drow = _prep_edges(
        cfg, inputs["x"], inputs["edge_index"], inputs["edge_attr"])
    wnames = ["Wl", "bl", "Wr", "br", "We", "att", "b_gat", "g1", "b1",
              "W_m1", "b_m1", "g_m", "b_m", "W_m2", "b_m2", "g2", "b2"]
    weights = {k: np.asarray(inputs[k], np.float32) for k in wnames}
    nc = build_trace(cfg, sched, weights)
    in_maps = _make_in_maps(cfg, inputs["x"], xph, xlo, ear, drelT, drow)
    return cfg, nc, in_maps


def kernel(**inputs) -> np.ndarray:
    cfg, nc, in_maps = build_all(inputs)
    res = run_bass_kernel_spmd(nc, in_maps, core_ids=list(range(cfg.NCORES)))
    out = np.concatenate(
        [r["y_out"][:cfg.NV] for r in res.results], axis=0
    ).astype(np.float32)
    return out


# revision 44
# speedup vs baseline: 1.5518x; 1.0017x over previous
"""Trainium2 Bass kernel for nn_NodeAttnModel (GATv2Conv + norm + MLP).

v2 architecture — no xl table, no dma_gather.

Key ideas:
  - Edges are sorted by destination and grouped into 80-node dst blocks;
    each 128-edge chunk belongs to one block.
  - The per-edge pre-activation  s = xl[src] + xr[dst] + ea@We  is produced
    by exactly TWO matmuls per chunk:
        MM1:  lhsT = Xp_hi (x[src].T rows 0:128, staged on host)  rhs = Wl_hi
        MM2:  lhsT = [Xp_lo(32); eaT(16); onehot_dst(80)]
              rhs  = [Wl_lo   ; We     ; xr_block     ]        (K = 128)
    The host supplies x[src] already permuted into edge order (it is a
    permutation of an *input*, so no on-device gather is needed).
  - Aggregation uses softmax linearity to avoid needing xl[src] per edge:
        agg = sum_e alpha*xl[src] = (S_s - S_ea@We5)/den - xr * den/(den+eps)
    where S_s = sum_e exp*s, S_ea = sum_e exp*ea (80 cols: 5 heads x 16),
    den = sum_e exp, all produced by ONE scatter matmul per chunk
    (rhs = [exp*s | exp*ea | exp], 245 cols, lhsT = dst one-hot).
  - Node phase: correction matmul + gating, then residual/LN/MLP as before.

All DMAs are large and batched (GROUP=8 tiles per call) to stay off the
descriptor-rate limits that dominated v1.
"""

import math

import numpy as np
import ml_dtypes

import concourse.bass as bass
import concourse.bacc as bacc
import concourse.mybir as mybir
import concourse.tile as tile
from concourse.bass_utils import run_bass_kernel_spmd

BF = ml_dtypes.bfloat16
F32 = mybir.dt.float32
BF16 = mybir.dt.bfloat16
AL = mybir.AluOpType
AF = mybir.ActivationFunctionType

# Problem constants
N, D, H, C, E, ED, HID = 50000, 160, 5, 32, 800000, 16, 512
EPS = 1e-5
SLOPE = 0.2
SELU_L = 1.0507009873554805
SELU_A = 1.6732632423543772

NCORES = 8
P = 128
BLK = 64           # dst nodes per block (mask rows 64:128 of the MM2 lhsT)
KLO = 48           # Xp_lo(32) + eaT(16) rows of the MM2 lhsT
KLOD = 64          # xlo DMA rows (48 data + 16 host zeros for K alignment)
CPT = 8            # chunks (of 128 edges) per tile
GROUP = 8          # tiles per DMA group
SENT = 1000.0      # dst_rel sentinel for padding edges
W_S = D            # wsea columns: [0:160) exp*s
W_EA = D + H * ED  # [160:240) exp*ea
W_DEN = W_EA + H   # [240:245) exp ; padded to 248
W_PAD = 248


class Cfg:
    def __init__(self, n=N, e=E, ncores=NCORES):
        self.N, self.E, self.NCORES = n, e, ncores
        self.NV = n // ncores                      # nodes per core
        self.NBLK = math.ceil(self.NV / BLK)       # dst blocks per core
        self.NPAD = self.NBLK * BLK                # block-padded nodes
        self.NT = math.ceil(self.NPAD / P)         # node-phase tiles
        self.NTP = self.NT * P                     # 128-padded nodes
        self.G = None                              # set after edge prep


def _prep_edges(cfg, x, edge_index, edge_attr):
    """Sort/pad edges, stage the permuted x[src] and edge data per core."""
    src = np.asarray(edge_index[0]).astype(np.int64)
    dst = np.asarray(edge_index[1]).astype(np.int64)
    e = src.shape[0]
    core = dst // cfg.NV
    rel = dst - core * cfg.NV
    blk = rel // BLK
    lane = rel - blk * BLK
    gkey = core * cfg.NBLK + blk
    order = np.argsort(gkey, kind="stable")
    gcounts = np.bincount(gkey, minlength=cfg.NCORES * cfg.NBLK)
    counts = gcounts.reshape(cfg.NCORES, cfg.NBLK)
    chunks_per = -(-counts.max(axis=0) // P)       # [NBLK]
    chunks_per[chunks_per == 0] = 1
    S = int(chunks_per.sum())
    T = -(-S // CPT)
    G = -(-T // GROUP)
    T = G * GROUP
    S_pad = T * CPT
    cfg.G = G

    chunk_blk = np.full(S_pad, cfg.NBLK - 1, np.int64)
    chunk_base = np.zeros(cfg.NBLK, np.int64)
    pos = 0
    for b in range(cfg.NBLK):
        chunk_base[b] = pos
        nch = int(chunks_per[b])
        chunk_blk[pos:pos + nch] = b
        pos += nch
    first_chunk = np.zeros(cfg.NBLK, np.int64)
    last_chunk = np.zeros(cfg.NBLK, np.int64)
    for b in range(cfg.NBLK):
        w = np.nonzero(chunk_blk == b)[0]
        first_chunk[b], last_chunk[b] = w[0], w[-1]

    gstart = np.zeros_like(gcounts)
    gstart[1:] = np.cumsum(gcounts)[:-1]
    ranks = np.arange(e) - gstart[gkey[order]]
    ecore = core[order]
    eslot = chunk_base[blk[order]] * P + ranks       # slot in [0, S_pad*P)

    SL = S_pad * P
    xbf = np.asarray(x, np.float32).astype(BF)
    ea32 = np.asarray(edge_attr, np.float32).astype(BF)

    Xflat = np.zeros((cfg.NCORES, SL, D), BF)
    EAflat = np.zeros((cfg.NCORES, SL, ED), BF)
    drel_flat = np.full((cfg.NCORES, SL), SENT, np.float32)
    Xflat[ecore, eslot] = xbf[src[order]]
    EAflat[ecore, eslot] = ea32[order]
    drel_flat[ecore, eslot] = lane[order].astype(np.float32)

    # [NC, G, 8192, D] -> feature-major per group
    Xg = Xflat.reshape(cfg.NCORES, G, GROUP * CPT * P, D)
    xph = np.ascontiguousarray(Xg[:, :, :, 0:P].transpose(0, 1, 3, 2))
    xlo = np.zeros((cfg.NCORES, G, KLOD, GROUP * CPT * P), BF)
    xlo[:, :, 0:D - P, :] = Xg[:, :, :, P:D].transpose(0, 1, 3, 2)
    EAg = EAflat.reshape(cfg.NCORES, G, GROUP * CPT * P, ED)
    xlo[:, :, D - P:KLO, :] = EAg.transpose(0, 1, 3, 2)
    # edge-major ea rows: [NC, G, 128, GROUP*CPT*ED]
    ear = np.ascontiguousarray(
        EAg.reshape(cfg.NCORES, G, GROUP * CPT, P, ED).transpose(0, 1, 3, 2, 4)
        .reshape(cfg.NCORES, G, P, GROUP * CPT * ED))
    dg = drel_flat.reshape(cfg.NCORES, G, GROUP * CPT, P)
    drelT = np.ascontiguousarray(
        dg.transpose(0, 1, 3, 2)).astype(BF)       # [NC, G, 128, G*CPT]
    drow = np.ascontiguousarray(
        dg.reshape(cfg.NCORES, G, 1, GROUP * CPT * P)).astype(BF)

    sched = dict(
        T=T, G=G,
        chunk_blk=chunk_blk.tolist(),
        first_chunk=first_chunk.tolist(),
        last_chunk=last_chunk.tolist(),
    )
    return sched, xph, xlo, ear, drelT, drow


def _nontriv(a, v):
    return not np.all(np.asarray(a) == v)


def build_trace(cfg, sched, weights):
    G = sched["G"]
    chunk_blk = sched["chunk_blk"]
    first_chunk_of = {g: b for b, g in enumerate(sched["first_chunk"])}
    last_chunk_of = {g: b for b, g in enumerate(sched["last_chunk"])}

    W = weights
    use_bl = _nontriv(W["bl"], 0.0)
    use_br = _nontriv(W["br"], 0.0)
    use_bgat = _nontriv(W["b_gat"], 0.0)
    use_g1 = _nontriv(W["g1"], 1.0)
    use_b1 = _nontriv(W["b1"], 0.0)
    use_bm1 = _nontriv(W["b_m1"], 0.0)
    use_gm = _nontriv(W["g_m"], 1.0)
    use_bm = _nontriv(W["b_m"], 0.0)
    use_bm2 = _nontriv(W["b_m2"], 0.0)
    use_g2 = _nontriv(W["g2"], 1.0)
    use_b2 = _nontriv(W["b2"], 0.0)

    nc = bacc.Bacc("TRN2", target_bir_lowering=False, debug=False)

    NBLK, NT, NTP = cfg.NBLK, cfg.NT, cfg.NTP
    EPG = GROUP * CPT * P          # edges per group (8192)
    CPG = GROUP * CPT              # chunks per group (64)

    # ---------------- I/O declarations ----------------
    d_xph = nc.dram_tensor("xph", [G, P, EPG], BF16, kind="ExternalInput")
    d_xlo = nc.dram_tensor("xlo", [G, KLOD, EPG], BF16, kind="ExternalInput")
    d_ear = nc.dram_tensor("ear", [G, P, CPG * ED], BF16, kind="ExternalInput")
    d_drelT = nc.dram_tensor("drelT", [G, P, CPG], BF16, kind="ExternalInput")
    d_drow = nc.dram_tensor("drow", [G, 1, EPG], BF16, kind="ExternalInput")
    d_xoT_hi = nc.dram_tensor("xoT_hi", [P, NTP], BF16, kind="ExternalInput")
    d_xoT_lo = nc.dram_tensor("xoT_lo", [D - P, NTP], BF16, kind="ExternalInput")
    d_xown = nc.dram_tensor("x_own", [NTP, D], F32, kind="ExternalInput")
    d_out = nc.dram_tensor("y_out", [NTP, D], F32, kind="ExternalOutput")

    def inline(arr, name):
        return nc.inline_tensor(np.ascontiguousarray(arr), name=name)

    bf = lambda a: np.asarray(a, np.float32).astype(BF)
    Wl = np.asarray(W["Wl"], np.float32)
    We = np.asarray(W["We"], np.float32)
    c_Wl_hi = inline(bf(Wl[0:P, :]), "c_Wl_hi")
    # MM2 rhs top 48 rows: [Wl_lo; We]
    rhs_lo = np.concatenate([Wl[P:D, :], We], axis=0)       # [48, 160]
    c_rhs_lo = inline(bf(rhs_lo), "c_rhs_lo")
    c_Wr_hi = inline(bf(W["Wr"][0:P, :]), "c_Wr_hi")
    c_Wr_lo = inline(bf(W["Wr"][P:D, :]), "c_Wr_lo")
    # We5: block-diagonal per-head We  [80, 160]
    We5 = np.zeros((H * ED, D), np.float32)
    for h in range(H):
        We5[h * ED:(h + 1) * ED, h * C:(h + 1) * C] = We[:, h * C:(h + 1) * C]
    c_We5 = inline(bf(We5), "c_We5")
    c_ident = inline(np.eye(P, dtype=BF), "c_ident")
    c_att = inline(np.tile(
        bf(np.asarray(W["att"]).reshape(1, D)), (P, 2)).copy(), "c_att")
    c_iota_c = inline(np.arange(BLK, dtype=np.float32).reshape(BLK, 1),
                      "c_iota_c")
    c_iota_r = inline(np.broadcast_to(
        np.arange(BLK, dtype=np.float32).reshape(1, BLK).astype(BF),
        (P, BLK)).copy(), "c_iota_r")
    c_Wm1_hi = inline(bf(W["W_m1"][0:P, :]), "c_Wm1_hi")
    c_Wm1_lo = inline(bf(W["W_m1"][P:D, :]), "c_Wm1_lo")
    c_Wm2 = inline(
        bf(W["W_m2"]).reshape(4, P, D).transpose(1, 0, 2).copy(), "c_Wm2")
    rows32 = lambda a: np.broadcast_to(
        np.asarray(a, np.float32).reshape(1, -1), (P, np.asarray(a).size)).copy()
    c_bl = inline(rows32(W["bl"]), "c_bl")
    c_br = inline(rows32(W["br"]), "c_br")
    c_bgat = inline(rows32(W["b_gat"]), "c_bgat")
    c_g1 = inline(rows32(W["g1"]), "c_g1")
    c_b1 = inline(rows32(W["b1"]), "c_b1")
    c_bm1 = inline(rows32(W["b_m1"]), "c_bm1")
    c_gm = inline(rows32(W["g_m"]), "c_gm")
    c_bm = inline(rows32(W["b_m"]), "c_bm")
    c_bm2 = inline(rows32(W["b_m2"]), "c_bm2")
    c_g2 = inline(rows32(W["g2"]), "c_g2")
    c_b2 = inline(rows32(W["b2"]), "c_b2")

    with tile.TileContext(nc) as tc:
        mesh_scope, _ = nc.enter_named_scope("ALGO_MESH", True)
        psp = tc.alloc_tile_pool(name="psp", bufs=8, space="PSUM")
        dram = tc.alloc_tile_pool(name="dram", bufs=1, space="DRAM")
        agg_d = dram.tile([NTP, W_PAD], F32, name="agg_d", tag="agg_d")
        xr_d = dram.tile([NTP, D], BF16, name="xr_d", tag="xr_d")

        cp = tc.alloc_tile_pool(name="consts", bufs=1)

        def csb(dr, shape, dtype, name):
            t = cp.tile(shape, dtype, name=name, tag=name)
            nc.sync.dma_start(out=t[tuple(slice(0, s) for s in shape)], in_=dr[:])
            return t

        Wl_hi = csb(c_Wl_hi, [P, D], BF16, "Wl_hi")
        rhs_lo_sb = csb(c_rhs_lo, [KLO, D], BF16, "rhs_lo_sb")
        Wr_hi = csb(c_Wr_hi, [P, D], BF16, "Wr_hi")
        Wr_lo = csb(c_Wr_lo, [D - P, D], BF16, "Wr_lo")
        We5_sb = csb(c_We5, [H * ED, D], BF16, "We5_sb")
        ident = csb(c_ident, [P, P], BF16, "ident")
        att_sb = csb(c_att, [P, 2 * D], BF16, "att_sb")
        iota_c = csb(c_iota_c, [BLK, 1], F32, "iota_c")
        iota_r = csb(c_iota_r, [P, BLK], BF16, "iota_r")
        Wm1_hi = csb(c_Wm1_hi, [P, HID], BF16, "Wm1_hi")
        Wm1_lo = csb(c_Wm1_lo, [D - P, HID], BF16, "Wm1_lo")
        Wm2_sb = csb(c_Wm2, [P, 4, D], BF16, "Wm2_sb")
        bl_sb = csb(c_bl, [P, D], F32, "bl_sb")
        br_sb = csb(c_br, [P, D], F32, "br_sb")
        bgat_sb = csb(c_bgat, [P, D], F32, "bgat_sb")
        g1_sb = csb(c_g1, [P, D], F32, "g1_sb")
        b1_sb = csb(c_b1, [P, D], F32, "b1_sb")
        bm1_sb = csb(c_bm1, [P, HID], F32, "bm1_sb")
        gm_sb = csb(c_gm, [P, HID], F32, "gm_sb")
        bm_sb = csb(c_bm, [P, HID], F32, "bm_sb")
        bm2_sb = csb(c_bm2, [P, D], F32, "bm2_sb")
        g2_sb = csb(c_g2, [P, D], F32, "g2_sb")
        b2_sb = csb(c_b2, [P, D], F32, "b2_sb")
        eps_sb = cp.tile([P, 1], F32, name="eps_sb", tag="eps_sb")
        nc.gpsimd.memset(eps_sb[:, :], float(EPS))
        lna_sb = cp.tile([P, 1], F32, name="lna_sb", tag="lna_sb")
        nc.gpsimd.memset(lna_sb[:, :], float(math.log(SELU_L * SELU_A)))

        # xoT kept resident: phase A (xr blocks) + phase C (xr recompute)
        xoT_hi = cp.tile([P, NTP], BF16, name="xoT_hi", tag="xoT_hi")
        nc.sync.dma_start(out=xoT_hi[:, :], in_=d_xoT_hi[:])
        xoT_lo = cp.tile([D - P, NTP], BF16, name="xoT_lo", tag="xoT_lo")
        nc.sync.dma_start(out=xoT_lo[:, :], in_=d_xoT_lo[:])

        # MM2 rhs per block: [Wl_lo; We; 0; xr_b]
        rhs_all = cp.tile([P, NBLK * D], BF16, name="rhs_all", tag="rhs_all")

        # ---------------- phase A: xr per block ----------------
        sc_xr, _ = nc.enter_named_scope("pA_xr", True)
        nc.gpsimd.memset(rhs_all[32:KLOD, :], 0.0)
        xr_sb = None
        for b in range(NBLK):
            nc.scalar.copy(out=rhs_all[0:KLO, b * D:(b + 1) * D],
                           in_=rhs_lo_sb[:, :])
            ps = psp.tile([BLK, D], F32, name=f"ps_xr{b}", tag="ps")
            cs = slice(b * BLK, (b + 1) * BLK)
            nc.tensor.matmul(ps[:, :], xoT_hi[:, cs], Wr_hi[:, :],
                             start=True, stop=False)
            nc.tensor.matmul(ps[:, :], xoT_lo[:, cs], Wr_lo[:, :],
                             start=False, stop=True)
            dst = rhs_all[KLOD:P, b * D:(b + 1) * D]
            if use_br:
                nc.vector.tensor_tensor(out=dst, in0=ps[:, :],
                                        in1=br_sb[0:BLK, :], op=AL.add)
            else:
                nc.vector.tensor_scalar_add(out=dst, in0=ps[:, :], scalar1=0.0)
            # stash xr rows for the node phase (incl. br if present)
            if xr_sb is None:
                xr_lo = b
                xr_sb = cp.tile([BLK, 8, D], BF16, name=f"xrsb{b}",
                                tag="xrsb", bufs=2)
            if use_br:
                nc.gpsimd.tensor_tensor(out=xr_sb[:, b - xr_lo, :],
                                        in0=ps[:, :], in1=br_sb[0:BLK, :],
                                        op=AL.add)
            else:
                nc.scalar.copy(out=xr_sb[:, b - xr_lo, :], in_=ps[:, :])
            if b - xr_lo == 7 or b == NBLK - 1:
                nb = b - xr_lo + 1
                nc.scalar.dma_start(
                    out=xr_d[xr_lo * BLK:(b + 1) * BLK, :]
                    .rearrange("(t l) w -> l t w", l=BLK),
                    in_=xr_sb[:, 0:nb, :])
                xr_sb = None
        nc.leave_named_scope("pA_xr", sc_xr, True)

        tc.strict_bb_all_engine_barrier()

        # ---------------- phase B: edges ----------------
        ep = tc.alloc_tile_pool(name="ep", bufs=1)
        sc_ed, _ = nc.enter_named_scope("pB_edge", True)
        # PE warm-up: sustained busy window pushes HAM to full clock
        prime = psp.tile([P, HID], F32, name="prime_b", tag="ps")
        for i in range(16):
            nc.tensor.matmul(prime[:, :], ident[:, :], Wm1_hi[:, :],
                             start=(i == 0), stop=(i == 15))
        agg_tiles = {}
        agg_sb = None
        agg_lo = 0
        for g in range(G):
            xph_t = ep.tile([P, EPG], BF16, name=f"xph{g}", tag="xph", bufs=2)
            comb_t = ep.tile([P, EPG], BF16, name=f"comb{g}", tag="comb", bufs=2)
            ear_t = ep.tile([P, CPG * ED], BF16, name=f"ear{g}", tag="ear",
                            bufs=2)
            drelT_t = ep.tile([P, CPG], BF16, name=f"drelT{g}", tag="drelT",
                              bufs=2)
            nc.sync.dma_start(out=xph_t[:, :], in_=d_xph[g])
            nc.sync.dma_start(out=comb_t[0:KLOD, :], in_=d_xlo[g])
            nc.sync.dma_start(out=ear_t[:, :], in_=d_ear[g])
            nc.sync.dma_start(out=drelT_t[:, :], in_=d_drelT[g])
            nc.sync.dma_start(out=comb_t[KLOD:P, :],
                              in_=d_drow[g].to_broadcast([BLK, EPG]))

            # one-hot masks: comb rows 64:128 (pick, in place), m2 (scatter)
            nc.vector.tensor_scalar(
                out=comb_t[KLOD:P, :], in0=comb_t[KLOD:P, :],
                scalar1=iota_c[:, 0:1], scalar2=None, op0=AL.is_equal)
            m2 = ep.tile([P, CPG, BLK], BF16, name=f"m2_{g}", tag="m2", bufs=2)
            nc.vector.tensor_tensor(
                out=m2[:, :, :],
                in0=iota_r[:, :].rearrange("p n -> p () n").to_broadcast(
                    [P, CPG, BLK]),
                in1=drelT_t[:, :].rearrange("p c -> p c ()").to_broadcast(
                    [P, CPG, BLK]),
                op=AL.is_equal)

            for t in range(GROUP):
                f_sb = ep.tile([P, CPT, D], BF16, name=f"f{g}_{t}", tag="f",
                               bufs=3)
                wsea = ep.tile([P, CPT, W_PAD], BF16, name=f"w{g}_{t}",
                               tag="w", bufs=3)
                sc_t = ep.tile([P, CPT, H], BF16, name=f"sc{g}_{t}", tag="sc",
                               bufs=3)
                for k in range(CPT // 2):
                    ks = slice(2 * k, 2 * k + 2)
                    ps_s = psp.tile([P, 2, D], F32, name=f"ps_s{g}_{t}_{k}",
                                    tag="ps")
                    for j in range(2):
                        c = t * CPT + 2 * k + j
                        gi = g * CPG + c
                        b = chunk_blk[gi]
                        es = slice(c * P, (c + 1) * P)
                        nc.tensor.matmul(ps_s[:, j, :], xph_t[:, es],
                                         Wl_hi[:, :], start=True, stop=False)
                        nc.tensor.matmul(ps_s[:, j, :], comb_t[:, es],
                                         rhs_all[:, b * D:(b + 1) * D],
                                         start=False, stop=True)
                    # f = leaky_relu(s), fused into the PSUM read
                    nc.scalar.activation(out=f_sb[:, ks, :],
                                         in_=ps_s[:, :, :], func=AF.Prelu,
                                         alpha=float(SLOPE))
                    # f *= att (in place), per-head reduce, exp
                    nc.vector.tensor_tensor(
                        out=f_sb[:, ks, :], in0=f_sb[:, ks, :],
                        in1=att_sb[:, :].rearrange("p (c f) -> p c f", c=2),
                        op=AL.mult)
                    with nc.allow_low_precision(
                            reason="bf16 score reduce, |score|<~3"):
                        nc.vector.tensor_reduce(
                            out=sc_t[:, ks, :],
                            in_=f_sb[:, ks, :].rearrange(
                                "p c (h z) -> p c h z", z=C),
                            axis=mybir.AxisListType.X, op=AL.add)
                    nc.scalar.activation(out=wsea[:, ks, W_EA:W_DEN],
                                         in_=sc_t[:, ks, :], func=AF.Exp)
                    expv = wsea[:, ks, W_EA:W_DEN]
                    nc.vector.tensor_tensor(
                        out=wsea[:, ks, 0:D].rearrange(
                            "p c (h z) -> p c h z", z=C),
                        in0=ps_s[:, :, :].rearrange(
                            "p c (h z) -> p c h z", z=C),
                        in1=expv.rearrange(
                            "p c h -> p c h ()").to_broadcast([P, 2, H, C]),
                        op=AL.mult)
                    nc.vector.tensor_tensor(
                        out=wsea[:, ks, W_S:W_EA].rearrange(
                            "p c (h z) -> p c h z", z=ED),
                        in0=ear_t[:, :].rearrange(
                            "p (c z) -> p c () z",
                            z=ED)[:, t * CPT + 2 * k:t * CPT + 2 * k + 2]
                        .to_broadcast([P, 2, H, ED]),
                        in1=expv.rearrange("p c h -> p c h ()").to_broadcast(
                            [P, 2, H, ED]),
                        op=AL.mult)

                    for j in range(2):
                        c = t * CPT + 2 * k + j
                        gi = g * CPG + c
                        b = chunk_blk[gi]
                        if gi in first_chunk_of:
                            agg_tiles[b] = psp.tile([BLK, W_PAD], F32,
                                                    name=f"agg{b}", tag="ps")
                        at = agg_tiles[b]
                        nc.tensor.matmul(at[:, :], m2[:, c, :], wsea[:, c - t * CPT, :],
                                         start=(gi in first_chunk_of),
                                         stop=(gi in last_chunk_of))
                        if gi in last_chunk_of:
                            if agg_sb is None:
                                agg_lo = b
                                agg_sb = ep.tile([BLK, 8, W_PAD], F32,
                                                 name=f"aggsb{b}", tag="aggsb",
                                                 bufs=2)
                            nc.scalar.copy(out=agg_sb[:, b - agg_lo, :],
                                           in_=at[:, :])
                            del agg_tiles[b]
                            if b - agg_lo == 7 or b == NBLK - 1:
                                nb = b - agg_lo + 1
                                nc.scalar.dma_start(
                                    out=agg_d[agg_lo * BLK:(b + 1) * BLK, :]
                                    .rearrange("(t l) w -> l t w", l=BLK),
                                    in_=agg_sb[:, 0:nb, :])
                                agg_sb = None
        nc.leave_named_scope("pB_edge", sc_ed, True)

        tc.strict_bb_all_engine_barrier()
        ep.release()

        # ---------------- phase C: node phase ----------------
        npo = tc.alloc_tile_pool(name="np", bufs=1)
        sc_nd, _ = nc.enter_named_scope("pC_node", True)
        prime2 = psp.tile([P, HID], F32, name="prime_c", tag="ps")
        for i in range(16):
            nc.tensor.matmul(prime2[:, :], ident[:, :], Wm1_hi[:, :],
                             start=(i == 0), stop=(i == 15))
        NTG = 8
        for j0 in range(0, NT, NTG):
            nt = min(NTG, NT - j0)
            rs = slice(j0 * P, (j0 + nt) * P)
            x_g = npo.tile([P, NTG, D], F32, name=f"x{j0}", tag="x", bufs=2)
            a_g = npo.tile([P, NTG, W_PAD], F32, name=f"a{j0}", tag="a",
                           bufs=2)
            xr_g = npo.tile([P, NTG, D], BF16, name=f"xr{j0}", tag="xr",
                            bufs=2)
            nc.sync.dma_start(
                out=x_g[:, 0:nt, :],
                in_=d_xown[rs, :].rearrange("(t p) d -> p t d", p=P))
            nc.sync.dma_start(
                out=a_g[:, 0:nt, :],
                in_=agg_d[rs, :].rearrange("(t p) d -> p t d", p=P))
            nc.sync.dma_start(
                out=xr_g[:, 0:nt, :],
                in_=xr_d[rs, :].rearrange("(t p) d -> p t d", p=P))
            y_g = npo.tile([P, NTG, D], F32, name=f"y{j0}", tag="y", bufs=2)
            t0_g = npo.tile([P, NTG, D], F32, name=f"t0g{j0}", tag="t0g",
                            bufs=2)
            o1_g = npo.tile([P, NTG, D], F32, name=f"o1g{j0}", tag="o1g",
                            bufs=2)
            u2_g = npo.tile([P, NTG, HID], BF16, name=f"u2g{j0}", tag="u2g",
                            bufs=2)
            mv_g = npo.tile([P, 3, NTG, 3], F32, name=f"mvg{j0}", tag="mvg",
                            bufs=2)

            def ln_stats(src_ap, ph, jj, nm):
                st = npo.tile([P, 6], F32, name=f"st{nm}", tag="st", bufs=4)
                nc.vector.bn_stats(out=st[:, :], in_=src_ap)
                nc.vector.bn_aggr(out=mv_g[:, ph, jj, 0:2], in_=st[:, :])

            def ln_batch_rstd(ph):
                # rstd for all tiles of the group in one sqrt + one recip
                sd = npo.tile([P, NTG], F32, name=f"sd{j0}_{ph}", tag="sd",
                              bufs=4)
                nc.scalar.activation(out=sd[:, :], in_=mv_g[:, ph, :, 1],
                                     func=AF.Sqrt, bias=eps_sb[:, 0:1])
                nc.vector.reciprocal(out=mv_g[:, ph, :, 2], in_=sd[:, :])

            def ln_apply(src_ap, out_ap, width, ph, jj, g_sb, b_sb,
                         use_g, use_b):
                nc.vector.scalar_tensor_tensor(
                    out=out_ap, in0=src_ap, scalar=mv_g[:, ph, jj, 0:1],
                    in1=mv_g[:, ph, jj, 2:3].to_broadcast([P, width]),
                    op0=AL.subtract, op1=AL.mult)
                if use_g:
                    nc.vector.tensor_tensor(out=out_ap, in0=out_ap,
                                            in1=g_sb[:, :], op=AL.mult)
                if use_b:
                    nc.gpsimd.tensor_tensor(out=out_ap, in0=out_ap,
                                            in1=b_sb[:, :], op=AL.add)

            # ---- stage 1: corr, gating, t0, LN1 stats ----
            for jj in range(nt):
                j = j0 + jj
                x_t = x_g[:, jj, :]
                a_t = a_g[:, jj, :]
                t0 = t0_g[:, jj, :]

                sea_b = npo.tile([P, H * ED], BF16, name=f"seab{j}",
                                 tag="seab", bufs=3)
                nc.scalar.copy(out=sea_b[:, :], in_=a_t[:, W_S:W_EA])
                ps_t = psp.tile([H * ED, P], BF16, name=f"ps_t{j}", tag="ps")
                nc.tensor.transpose(out=ps_t[:, :], in_=sea_b[:, :],
                                    identity=ident[:, :])
                sea_T = npo.tile([H * ED, P], BF16, name=f"seaT{j}",
                                 tag="seaT", bufs=3)
                nc.scalar.copy(out=sea_T[:, :], in_=ps_t[:, :])
                ps_co = psp.tile([P, D], F32, name=f"ps_co{j}", tag="ps")
                nc.tensor.matmul(ps_co[:, :], sea_T[:, :], We5_sb[:, :],
                                 start=True, stop=True)

                den = npo.tile([P, H], F32, name=f"den{j}", tag="den", bufs=3)
                nc.vector.tensor_scalar_add(out=den[:, :],
                                            in0=a_t[:, W_EA:W_DEN],
                                            scalar1=1e-30)
                rec = npo.tile([P, 2, H], F32, name=f"rec{j}", tag="rec",
                               bufs=3)
                nc.vector.reciprocal(out=rec[:, 0, :], in_=den[:, :])
                nc.vector.tensor_tensor(out=rec[:, 1, :],
                                        in0=a_t[:, W_EA:W_DEN],
                                        in1=rec[:, 0, :], op=AL.mult)

                nc.vector.tensor_tensor(out=t0, in0=a_t[:, 0:D],
                                        in1=ps_co[:, :], op=AL.subtract)
                nc.vector.tensor_tensor(
                    out=t0.rearrange("p (h z) -> p h z", z=C),
                    in0=t0.rearrange("p (h z) -> p h z", z=C),
                    in1=rec[:, 0, :].rearrange("p h -> p h ()").to_broadcast(
                        [P, H, C]),
                    op=AL.mult)
                xrg = npo.tile([P, D], F32, name=f"xrg{j}", tag="xrg", bufs=3)
                nc.vector.tensor_tensor(
                    out=xrg[:, :].rearrange("p (h z) -> p h z", z=C),
                    in0=xr_g[:, jj, :].rearrange("p (h z) -> p h z", z=C),
                    in1=rec[:, 1, :].rearrange("p h -> p h ()").to_broadcast(
                        [P, H, C]),
                    op=AL.mult)
                nc.vector.tensor_tensor(out=t0, in0=t0, in1=xrg[:, :],
                                        op=AL.subtract)
                nc.vector.tensor_tensor(out=t0, in0=x_t, in1=t0, op=AL.add)
                if use_bl:
                    nc.gpsimd.tensor_tensor(out=t0, in0=t0,
                                            in1=bl_sb[:, :], op=AL.add)
                if use_bgat:
                    nc.gpsimd.tensor_tensor(out=t0, in0=t0,
                                            in1=bgat_sb[:, :], op=AL.add)
                ln_stats(t0, 0, jj, f"1_{j}")
            ln_batch_rstd(0)

            # ---- stage 2: LN1 apply, mm1, selu, LN2 stats ----
            for jj in range(nt):
                j = j0 + jj
                out1 = o1_g[:, jj, :]
                ln_apply(t0_g[:, jj, :], out1, D, 0, jj, g1_sb, b1_sb,
                         use_g1, use_b1)
                out1b = npo.tile([P, D], BF16, name=f"o1b{j}", tag="o1b",
                                 bufs=3)
                nc.scalar.copy(out=out1b[:, :], in_=out1)

                pt0 = psp.tile([P, P], BF16, name=f"pt0_{j}", tag="ps")
                nc.tensor.transpose(out=pt0[:, :], in_=out1b[:, 0:P],
                                    identity=ident[:, :])
                t0s = npo.tile([P, P], BF16, name=f"t0s{j}", tag="t0s", bufs=3)
                nc.scalar.copy(out=t0s[:, :], in_=pt0[:, :])
                pt1 = psp.tile([D - P, P], BF16, name=f"pt1_{j}", tag="ps")
                nc.tensor.transpose(out=pt1[:, :], in_=out1b[:, P:D],
                                    identity=ident[:, :])
                t1s = npo.tile([D - P, P], BF16, name=f"t1s{j}", tag="t1s",
                               bufs=3)
                nc.scalar.copy(out=t1s[:, :], in_=pt1[:, :])
                ps_h = psp.tile([P, HID], F32, name=f"ps_h{j}", tag="ps")
                nc.tensor.matmul(ps_h[:, :], t0s[:, :], Wm1_hi[:, :],
                                 start=True, stop=False)
                nc.tensor.matmul(ps_h[:, :], t1s[:, :], Wm1_lo[:, :],
                                 start=False, stop=True)

                if use_bm1:
                    y_sb = npo.tile([P, HID], F32, name=f"ysb{j}", tag="ysb",
                                    bufs=2)
                    nc.vector.tensor_tensor(out=y_sb[:, :], in0=ps_h[:, :],
                                            in1=bm1_sb[:, :], op=AL.add)
                    ysrc = y_sb[:, :]
                else:
                    ysrc = ps_h[:, :]
                e_sb = npo.tile([P, HID], BF16, name=f"esb{j}", tag="esb",
                                bufs=3)
                nc.scalar.activation(out=e_sb[:, :], in_=ysrc, func=AF.Exp,
                                     bias=lna_sb[:, 0:1])
                r_sb = npo.tile([P, HID], BF16, name=f"rsb{j}", tag="rsb",
                                bufs=3)
                nc.scalar.activation(out=r_sb[:, :], in_=ysrc, func=AF.Relu,
                                     scale=float(SELU_L))
                u2 = u2_g[:, jj, :]
                nc.vector.scalar_tensor_tensor(
                    out=u2, in0=e_sb[:, :], scalar=float(SELU_L * SELU_A),
                    in1=r_sb[:, :], op0=AL.min, op1=AL.add)
                ln_stats(u2, 1, jj, f"2_{j}")
            ln_batch_rstd(1)

            # ---- stage 3: LN2 apply, mm2, residual, LN3 stats ----
            for jj in range(nt):
                j = j0 + jj
                h_bf = npo.tile([P, HID], BF16, name=f"hbf{j}", tag="hbf",
                                bufs=3)
                ln_apply(u2_g[:, jj, :], h_bf[:, :], HID, 1, jj, gm_sb, bm_sb,
                         use_gm, use_bm)

                ps_m = psp.tile([P, D], F32, name=f"ps_m{j}", tag="ps")
                for k in range(4):
                    pth = psp.tile([P, P], BF16, name=f"pth{j}_{k}", tag="ps")
                    nc.tensor.transpose(out=pth[:, :],
                                        in_=h_bf[:, k * P:(k + 1) * P],
                                        identity=ident[:, :])
                    hts = npo.tile([P, P], BF16, name=f"hts{j}_{k}", tag="hts",
                                   bufs=4)
                    nc.scalar.copy(out=hts[:, :], in_=pth[:, :])
                    nc.tensor.matmul(ps_m[:, :], hts[:, :], Wm2_sb[:, k, :],
                                     start=(k == 0), stop=(k == 3))

                t2 = t0_g[:, jj, :]
                nc.vector.tensor_tensor(out=t2, in0=o1_g[:, jj, :],
                                        in1=ps_m[:, :], op=AL.add)
                if use_bm2:
                    nc.gpsimd.tensor_tensor(out=t2, in0=t2,
                                            in1=bm2_sb[:, :], op=AL.add)
                ln_stats(t2, 2, jj, f"3_{j}")
            ln_batch_rstd(2)

            # ---- stage 4: LN3 apply -> y ----
            for jj in range(nt):
                ln_apply(t0_g[:, jj, :], y_g[:, jj, :], D, 2, jj, g2_sb,
                         b2_sb, use_g2, use_b2)
            nc.scalar.dma_start(
                out=d_out[rs, :].rearrange("(t p) d -> p t d", p=P),
                in_=y_g[:, 0:nt, :])
        nc.leave_named_scope("pC_node", sc_nd, True)

        npo.release()
        psp.release()
        dram.release()
        cp.release()
        nc.leave_named_scope("ALGO_MESH", mesh_scope, True)

    nc.compile()
    return nc


def _make_in_maps(cfg, x, xph, xlo, ear, drelT, drow):
    x32 = np.asarray(x, np.float32)
    in_maps = []
    for k in range(cfg.NCORES):
        xo = np.zeros((cfg.NTP, D), np.float32)
        xo[:cfg.NV] = x32[k * cfg.NV:(k + 1) * cfg.NV]
        xoT = np.zeros((D, cfg.NTP), BF)
        xoT[:, :cfg.NV] = x32[k * cfg.NV:(k + 1) * cfg.NV].astype(BF).T
        in_maps.append({
            "xph": xph[k], "xlo": xlo[k], "ear": ear[k],
            "drelT": drelT[k], "drow": drow[k],
            "xoT_hi": np.ascontiguousarray(xoT[0:P]),
            "xoT_lo": np.ascontiguousarray(xoT[P:D]),
            "x_own": xo,
        })
    return in_maps


def build_all(inputs, cfg=None):
    cfg = cfg or Cfg()
    sched, xph, xlo, ear, drelT, drow = _prep_edges(
        cfg, inputs["x"], inputs["edge_index"], inputs["edge_attr"])
    wnames = ["Wl", "bl", "Wr", "br", "We", "att", "b_gat", "g1", "b1",
              "W_m1", "b_m1", "g_m", "b_m", "W_m2", "b_m2", "g2", "b2"]
    weights = {k: np.asarray(inputs[k], np.float32) for k in wnames}
    nc = build_trace(cfg, sched, weights)
    in_maps = _make_in_maps(cfg, inputs["x"], xph, xlo, ear, drelT, drow)
    return cfg, nc, in_maps


def kernel(**inputs) -> np.ndarray:
    cfg, nc, in_maps = build_all(inputs)
    res = run_bass_kernel_spmd(nc, in_maps, core_ids=list(range(cfg.NCORES)))
    out = np.concatenate(
        [r["y_out"][:cfg.NV] for r in res.results], axis=0
    ).astype(np.float32)
    return out
